# revision 1
# baseline (speedup 1.0000x reference)
"""Trainium2 Bass kernel for MllamaTextSdpaAttention (GQA + RoPE + causal SDPA).

Strategy: tensor-parallel over heads across 8 NeuronCores. Core c owns
q-heads [4c, 4c+4) and kv-head c (kv groups intact). Each core computes
hidden @ Wq/Wk/Wv slices, RoPE, causal attention for its heads, and its
row-slice of the Wo matmul, yielding a partial [T, DIM] output (bf16).
The host sums the 8 partials in f32.

Layout tricks:
- hidden_states is fed transposed ([DIM, T], bf16) so every projection
  matmul has the contraction dim (features) on partitions.
- Q/K projections produce Q^T/K^T directly (head_dim=128 on partitions).
- The RoPE even/odd pairing is de-interleaved by permuting Wq/Wk columns
  on the host, turning RoPE into a half-rotation: the partner element sits
  64 partitions away, reachable with plain partition-offset slices. The
  d-permutation cancels in q.k^T. The 1/sqrt(d) scale is folded into Q's
  cos/sin tables.
- Scores are computed TRANSPOSED: scT[k, q] = K_rot^T(tile).T @ Q_rot^T.
  exp(scT) is then directly the moving operand for the P@V matmul
  (out^T[d,q] = V[k,d].T @ expT[k,q]) -- no P transposes or PSUM->SBUF
  P copies. Softmax denominators come from a ones-vector matmul on the
  PE (sum over k = partition dim), and the 1/sum normalization is applied
  to the small out^T tile (via a PE-broadcast of the reciprocal row),
  not to P. No max-subtraction: scores are bounded (|s| <= ~20) so f32
  exp is safe, and masked entries use the additive -1e9 mask -> exp = 0.
- Causality at 128-block granularity: k-blocks strictly above the
  diagonal are never computed or read; diagonal blocks get the transposed
  additive mask from the actual attention_mask input.
- The 1/rowsum reciprocal row is broadcast across partitions on the idle
  GpSimd engine (partition_broadcast), and each group's normalization
  epilogue is deferred into the next group (software pipelining) so the
  PE never waits on the DVE reciprocal.
- Emission interleaves projection chunks with the attention groups they
  unblock (chunk0 -> b0/qb0 groups -> chunk1 -> b0/qb1 groups -> ...) and
  all [128,512]-f32 PSUM scratch (projection accumulators, score tiles,
  output accumulators) shares one 5-slot pool (+2 ot +1 rs = 8 banks)
  so the whole kernel fits PSUM without phase barriers.
- TimelineSim (instruction cost model): ~353 us/core; PE busy ~326 us
  (92% occupancy), which is the bf16 matmul-column floor for this
  decomposition.
"""

import numpy as np
import ml_dtypes

import concourse.bacc as bacc
import concourse.bass as bass
import concourse.mybir as mybir
from concourse.tile import TileContext
from concourse import bass_utils

BF16 = mybir.dt.bfloat16
F32 = mybir.dt.float32

B, S, DIM = 2, 1024, 4096
T = B * S                     # 2048 tokens, batch-major
N_HEADS, N_KV = 32, 8
HD = 128                      # head dim == partition count
N_CORES = 8
HL = N_HEADS // N_CORES       # 4 local q-heads per core
KT = DIM // 128               # 32 feature tiles
CH = 512                      # projection token-chunk
NCHUNK = T // CH
QB = 512                      # attention q-block width
TT = T // 128                 # 16 token tiles global
SCALE = 1.0 / float(np.sqrt(HD))

_CACHE: dict = {}


def _build():
    nc = bacc.Bacc("TRN2", target_bir_lowering=False, debug=False,
                   enable_asserts=False)

    hsT = nc.dram_tensor("hsT", [DIM, T], BF16, kind="ExternalInput")
    wq = nc.dram_tensor("wq", [DIM, HL * HD], BF16, kind="ExternalInput")
    wk = nc.dram_tensor("wk", [DIM, HD], BF16, kind="ExternalInput")
    wv = nc.dram_tensor("wv", [DIM, HD], BF16, kind="ExternalInput")
    wo = nc.dram_tensor("wo", [HL * HD, DIM], BF16, kind="ExternalInput")
    cos_q = nc.dram_tensor("cos_q", [HD, T], BF16, kind="ExternalInput")
    sin_q = nc.dram_tensor("sin_q", [HD, T], BF16, kind="ExternalInput")
    cos_k = nc.dram_tensor("cos_k", [HD, T], BF16, kind="ExternalInput")
    sin_k = nc.dram_tensor("sin_k", [HD, T], BF16, kind="ExternalInput")
    maskT = nc.dram_tensor("maskT", [128, 128], F32, kind="ExternalInput")
    out = nc.dram_tensor("out", [T, DIM], BF16, kind="ExternalOutput")

    Exp = mybir.ActivationFunctionType.Exp

    with TileContext(nc) as tc:
        with tc.tile_pool(name="consts", bufs=1) as cpool, \
             tc.tile_pool(name="hs", bufs=2) as hpool, \
             tc.tile_pool(name="rope_tmp", bufs=2) as rpool, \
             tc.tile_pool(name="work_ps", bufs=5, space=bass.MemorySpace.PSUM) as wpool, \
             tc.tile_pool(name="ot_ps", bufs=2, space=bass.MemorySpace.PSUM) as otpool, \
             tc.tile_pool(name="rs_ps", bufs=1, space=bass.MemorySpace.PSUM) as rspool, \
             tc.tile_pool(name="et", bufs=6) as epool, \
             tc.tile_pool(name="bc_sb", bufs=2) as bcsbpool, \
             tc.tile_pool(name="recip", bufs=4) as rcpool, \
             tc.tile_pool(name="out_sb", bufs=6) as xsbpool:

            wq_h = [cpool.tile([128, KT, HD], BF16, tag=f"wq{m}", name=f"wq{m}")
                    for m in range(HL)]
            wk_sb = cpool.tile([128, KT, HD], BF16, tag="wk")
            wv_sb = cpool.tile([128, KT, HD], BF16, tag="wv")
            cq_sb = cpool.tile([128, T], BF16, tag="cq")
            sq_sb = cpool.tile([128, T], BF16, tag="sq")
            ck_sb = cpool.tile([128, T], BF16, tag="ck")
            sk_sb = cpool.tile([128, T], BF16, tag="sk")
            maskT_sb = cpool.tile([128, 128], F32, tag="maskT")
            ones_k = cpool.tile([128, 1], BF16, tag="ones_k")
            qt_rot = cpool.tile([128, HL, T], BF16, tag="qt")
            kt_rot = cpool.tile([128, T], BF16, tag="kt")
            v_sb = cpool.tile([128, TT, HD], BF16, tag="v")
            ao = cpool.tile([128, HL, T], BF16, tag="ao")

            wq_r = wq.ap().rearrange("(kt p) n -> p kt n", p=128)
            hsT_r = hsT.ap().rearrange("(kt p) t -> p kt t", p=128)

            # startup-critical DMA first: the k-tiles the first matmuls touch
            nc.sync.dma_start(wq_h[0][:, 0:8, :], wq_r[:, 0:8, 0:HD])
            nc.sync.dma_start(wq_h[0][:, 8:KT, :], wq_r[:, 8:KT, 0:HD])

            def late_consts():
                nc.sync.dma_start(wq_h[1], wq_r[:, :, HD:2 * HD])
                nc.sync.dma_start(cq_sb, cos_q.ap())
                nc.sync.dma_start(sq_sb, sin_q.ap())
                for m in range(2, HL):
                    nc.sync.dma_start(wq_h[m], wq_r[:, :, m * HD:(m + 1) * HD])
                nc.sync.dma_start(wk_sb, wk.ap().rearrange("(kt p) n -> p kt n", p=128))
                nc.sync.dma_start(ck_sb, cos_k.ap())
                nc.sync.dma_start(sk_sb, sin_k.ap())
                nc.sync.dma_start(wv_sb, wv.ap().rearrange("(kt p) n -> p kt n", p=128))
                nc.sync.dma_start(maskT_sb, maskT.ap())
                nc.vector.memset(ones_k, 1.0)

            def rope(ps, out_ap, cos_ap, sin_ap):
                """out = ps*cos + halfswap(ps)*sin  (signs baked into sin)."""
                t1 = rpool.tile([128, CH], F32, tag="r1", name="t1")
                t2 = rpool.tile([128, CH], F32, tag="r2", name="t2")
                nc.vector.tensor_mul(t1, ps, cos_ap)
                nc.vector.tensor_mul(t2[0:64, :], ps[64:128, :], sin_ap[0:64, :])
                nc.vector.tensor_mul(t2[64:128, :], ps[0:64, :], sin_ap[64:128, :])
                nc.vector.tensor_add(out_ap, t1, t2)

            def emit_chunk(c):
                t0 = c * CH
                hs_sb = hpool.tile([128, KT, CH], BF16, tag="hs", name="hs_sb")
                for g in range(4):
                    nc.sync.dma_start(hs_sb[:, g * 8:(g + 1) * 8, :],
                                      hsT_r[:, g * 8:(g + 1) * 8, t0:t0 + CH])
                for m in range(HL):
                    ps = wpool.tile([128, CH], F32, tag="work", name="ps_q")
                    for kt in range(KT):
                        nc.tensor.matmul(ps, wq_h[m][:, kt, :], hs_sb[:, kt, :],
                                         start=(kt == 0), stop=(kt == KT - 1))
                    if c == 0 and m == 0:
                        late_consts()
                    rope(ps, qt_rot[:, m, t0:t0 + CH],
                         cq_sb[:, t0:t0 + CH], sq_sb[:, t0:t0 + CH])
                ps = wpool.tile([128, CH], F32, tag="work", name="ps_k")
                for kt in range(KT):
                    nc.tensor.matmul(ps, wk_sb[:, kt, :], hs_sb[:, kt, :],
                                     start=(kt == 0), stop=(kt == KT - 1))
                rope(ps, kt_rot[:, t0:t0 + CH],
                     ck_sb[:, t0:t0 + CH], sk_sb[:, t0:t0 + CH])
                for vi in range(CH // 128):
                    tt = t0 // 128 + vi
                    ps = wpool.tile([128, HD], F32, tag="work", name="ps_v")
                    for kt in range(KT):
                        nc.tensor.matmul(ps, hs_sb[:, kt, vi * 128:(vi + 1) * 128],
                                         wv_sb[:, kt, :],
                                         start=(kt == 0), stop=(kt == KT - 1))
                    nc.scalar.copy(v_sb[:, tt, :], ps)

            # --- attention group machinery (transposed-scores scheme) ---
            pending = [None]

            def epilogue(st):
                rs, ot, h, q0 = st
                recip = rcpool.tile([1, QB], F32, tag="recip", name="recip")
                nc.vector.reciprocal(recip, rs)
                bcs = bcsbpool.tile([128, QB], F32, tag="bcs", name="bcs")
                nc.gpsimd.partition_broadcast(bcs, recip)
                nc.vector.tensor_mul(ao[:, h, q0:q0 + QB], ot, bcs)

            def emit_group(b, h, qb):
                q0 = b * S + qb * QB
                n_kt = (qb + 1) * (QB // 128)
                rs = rspool.tile([1, QB], F32, tag="rs", name="rs")
                ot = otpool.tile([128, QB], F32, tag="ot", name="ot")
                ets = [None] * n_kt

                def emit_sc(kt):
                    c0 = max(0, kt - qb * (QB // 128)) * 128
                    sc = wpool.tile([128, QB], F32, tag="work", name="sc")
                    nc.tensor.matmul(
                        sc[:, c0:],
                        kt_rot[:, b * S + kt * 128:b * S + (kt + 1) * 128],
                        qt_rot[:, h, q0 + c0:q0 + QB],
                        start=True, stop=True)
                    jd = kt - qb * (QB // 128)
                    if 0 <= jd < QB // 128:
                        nc.vector.tensor_add(sc[:, jd * 128:(jd + 1) * 128],
                                             sc[:, jd * 128:(jd + 1) * 128],
                                             maskT_sb)
                    et = epool.tile([128, QB], BF16, tag="et", name="et")
                    nc.scalar.activation(et[:, c0:], sc[:, c0:], Exp,
                                         bias=0.0, scale=1.0)
                    ets[kt] = (et, c0)

                for w in range(min(4, n_kt)):
                    emit_sc(w)
                for kt in range(n_kt):
                    if kt + 4 < n_kt:
                        emit_sc(kt + 4)
                    et, c0 = ets[kt]
                    nc.tensor.matmul(rs[:, c0:], ones_k, et[:, c0:],
                                     start=(kt == 0), stop=(kt == n_kt - 1))
                    nc.tensor.matmul(ot[:, c0:], v_sb[:, b * (S // 128) + kt, :],
                                     et[:, c0:], start=(kt == 0),
                                     stop=(kt == n_kt - 1))
                    ets[kt] = None
                    if kt == 0 and pending[0] is not None:
                        epilogue(pending[0])
                        pending[0] = None
                pending[0] = (rs, ot, h, q0)

            # --- interleaved emission: each chunk unblocks a set of groups ---
            # chunk c covers tokens [c*512, (c+1)*512) = batch c//2, q-block c%2
            wo_sb = None
            for c in range(NCHUNK):
                emit_chunk(c)
                b, qb = c // 2, c % 2
                for h in range(HL):
                    emit_group(b, h, qb)
                if c == NCHUNK - 1:
                    # wo reuses an hs slot (same size); DMA overlaps the
                    # final attention groups
                    wo_sb = hpool.tile([128, HL, DIM], BF16, tag="hs",
                                       name="wo_sb")
                    nc.sync.dma_start(
                        wo_sb, wo.ap().rearrange("(kh p) n -> p kh n", p=128))
            if pending[0] is not None:
                epilogue(pending[0])
                pending[0] = None

            # ---- output projection (row-parallel Wo) ----
            for tt in range(TT):
                for ni, n0 in enumerate(range(0, DIM, 512)):
                    ps = wpool.tile([128, 512], F32, tag="work", name="ps_o")
                    for kh in range(HL):
                        nc.tensor.matmul(ps, ao[:, kh, tt * 128:(tt + 1) * 128],
                                         wo_sb[:, kh, n0:n0 + 512],
                                         start=(kh == 0), stop=(kh == HL - 1))
                    osb = xsbpool.tile([128, 512], BF16, tag="osb", name="osb")
                    if (tt * 8 + ni) % 2 == 0:
                        nc.scalar.copy(osb, ps)
                    else:
                        nc.vector.tensor_copy(osb, ps)
                    nc.sync.dma_start(out.ap()[tt * 128:(tt + 1) * 128,
                                               n0:n0 + 512], osb)
    nc.compile()
    return nc


def _get_nc():
    if "nc" not in _CACHE:
        _CACHE["nc"] = _build()
    return _CACHE["nc"]


def _prep_inputs(inputs) -> list[dict]:
    bf16 = ml_dtypes.bfloat16
    hs = np.asarray(inputs["hidden_states"], dtype=np.float32).reshape(T, DIM)
    hsT = np.ascontiguousarray(hs.T).astype(bf16)

    fc = np.asarray(inputs["freqs_cos"], dtype=np.float32).reshape(T, HD // 2).T
    fs = np.asarray(inputs["freqs_sin"], dtype=np.float32).reshape(T, HD // 2).T
    cos2 = np.concatenate([fc, fc], axis=0)            # [128, T]
    sin2 = np.concatenate([-fs, fs], axis=0)           # signed half-rotation
    cos_qv = np.ascontiguousarray(cos2 * SCALE).astype(bf16)
    sin_qv = np.ascontiguousarray(sin2 * SCALE).astype(bf16)
    cos_kv = np.ascontiguousarray(cos2).astype(bf16)
    sin_kv = np.ascontiguousarray(sin2).astype(bf16)

    maskT = np.ascontiguousarray(
        np.asarray(inputs["attention_mask"], dtype=np.float32)[0, 0, :128, :128].T)

    perm = np.concatenate([np.arange(0, HD, 2), np.arange(1, HD, 2)])
    Wq = np.asarray(inputs["Wq"], dtype=np.float32)
    Wk = np.asarray(inputs["Wk"], dtype=np.float32)
    Wv = np.asarray(inputs["Wv"], dtype=np.float32)
    Wo = np.asarray(inputs["Wo"], dtype=np.float32)

    in_maps = []
    for c in range(N_CORES):
        wq_c = np.concatenate(
            [Wq[:, (c * HL + h) * HD:(c * HL + h + 1) * HD][:, perm]
             for h in range(HL)], axis=1)
        wk_c = Wk[:, c * HD:(c + 1) * HD][:, perm]
        wv_c = Wv[:, c * HD:(c + 1) * HD]
        wo_c = Wo[c * HL * HD:(c + 1) * HL * HD, :]
        in_maps.append({
            "hsT": hsT,
            "wq": np.ascontiguousarray(wq_c).astype(bf16),
            "wk": np.ascontiguousarray(wk_c).astype(bf16),
            "wv": np.ascontiguousarray(wv_c).astype(bf16),
            "wo": np.ascontiguousarray(wo_c).astype(bf16),
            "cos_q": cos_qv, "sin_q": sin_qv,
            "cos_k": cos_kv, "sin_k": sin_kv,
            "maskT": maskT,
        })
    return in_maps


def kernel(**inputs) -> np.ndarray:
    nc = _get_nc()
    in_maps = _prep_inputs(inputs)
    res = bass_utils.run_bass_kernel_spmd(nc, in_maps,
                                          core_ids=list(range(N_CORES)))
    acc = np.zeros((T, DIM), dtype=np.float32)
    for c in range(N_CORES):
        acc += np.asarray(res.results[c]["out"], dtype=np.float32)
    return acc.reshape(B, S, DIM)



# revision 2
# speedup vs baseline: 1.1692x; 1.1692x over previous
"""Trainium2 Bass kernel for MllamaTextSdpaAttention (GQA + RoPE + causal SDPA).

Strategy: tensor-parallel over heads across 8 NeuronCores. Core c owns
q-heads [4c, 4c+4) and kv-head c (kv groups intact). Each core computes
hidden @ Wq/Wk/Wv slices, RoPE, causal attention for its heads, and its
row-slice of the Wo matmul, yielding a partial [T, DIM] output (bf16).
The host sums the 8 partials in f32.

Layout tricks (see kernel_baseline for the bf16 ancestor):
- hidden_states is fed transposed ([DIM, T]) so every projection matmul
  has the contraction dim (features) on partitions.
- All four projections (Q/K/V/O) run on the PE in fp8e4m3 DoubleRow mode
  (2 k-tiles of contraction per instruction at 0.5 cycles/column = 4x the
  bf16 FLOP rate). Accuracy is preserved with a 3-term residual split:
  each operand X is split on the host (or on-device for ao) into
  Xh = fp8(X), Xl = fp8(X - Xh), and W@X ~= Wh@Xh + Wl@Xh + Wh@Xl
  (the dropped Wl@Xl term is ~0.1% relative). Net cost: 0.75x the bf16
  column count for the projections. Weights are pre-scaled on the host
  (x32 for Wq/Wk/Wo, x16 for Wv) to sit in e4m3's normal range; the
  descales fold into the RoPE cos/sin tables and the host-side gather.
- Q/K projections produce Q^T/K^T directly (head_dim=128 on partitions).
- RoPE even/odd pairing is de-interleaved by permuting Wq/Wk columns on
  the host, turning RoPE into a half-rotation (partner element 64
  partitions away). The d-permutation cancels in q.k^T. The 1/sqrt(d)
  scale (and 1/32 weight descale) is folded into the cos/sin tables.
- Attention stays bf16: scores are computed TRANSPOSED (scT = K_rot^T.T @
  Q_rot^T), exp(scT) feeds the P@V matmul directly, softmax denominators
  come from a ones-vector matmul, and the 1/rowsum normalization applies
  to the small out^T tile via a GpSimd partition-broadcast. No
  max-subtraction (scores bounded, f32 exp). Causality at 128-block
  granularity.
- The normalized out^T tile is split on-device into ao_h/ao_l fp8 pairs
  (Act copy + DVE subtract) feeding the fp8 O projection.
- Emission interleaves projection chunks with the attention groups they
  unblock; all [128,512]-f32 PSUM scratch shares one 5-slot pool
  (+2 ot +1 rs = 8 banks).
"""

import numpy as np
import ml_dtypes

import concourse.bacc as bacc
import concourse.bass as bass
import concourse.mybir as mybir
from concourse.tile import TileContext
from concourse import bass_utils

BF16 = mybir.dt.bfloat16
F32 = mybir.dt.float32
F8 = mybir.dt.float8e4
E4M3 = ml_dtypes.float8_e4m3

B, S, DIM = 2, 1024, 4096
T = B * S                     # 2048 tokens, batch-major
N_HEADS, N_KV = 32, 8
HD = 128                      # head dim == partition count
N_CORES = 8
HL = N_HEADS // N_CORES       # 4 local q-heads per core
KT = DIM // 128               # 32 feature tiles
KP = KT // 2                  # 16 k-tile PAIRS (DoubleRow)
CH = 512                      # projection token-chunk
NCHUNK = T // CH
QB = 512                      # attention q-block width
TT = T // 128                 # 16 token tiles global
SCALE = 1.0 / float(np.sqrt(HD))
S_Q = 32.0                    # weight pre-scales for fp8 range
S_K = 32.0
S_V = 16.0
S_O = 32.0
DR = mybir.MatmulPerfMode.DoubleRow

_CACHE: dict = {}


def _build():
    nc = bacc.Bacc("TRN2", target_bir_lowering=False, debug=False,
                   enable_asserts=False)

    hsh_d = nc.dram_tensor("hsh", [DIM, T], F8, kind="ExternalInput")
    hsl_d = nc.dram_tensor("hsl", [DIM, T], F8, kind="ExternalInput")
    wqh_d = nc.dram_tensor("wqh", [DIM, HL * HD], F8, kind="ExternalInput")
    wql_d = nc.dram_tensor("wql", [DIM, HL * HD], F8, kind="ExternalInput")
    wkh_d = nc.dram_tensor("wkh", [DIM, HD], F8, kind="ExternalInput")
    wkl_d = nc.dram_tensor("wkl", [DIM, HD], F8, kind="ExternalInput")
    wvh_d = nc.dram_tensor("wvh", [DIM, HD], F8, kind="ExternalInput")
    wvl_d = nc.dram_tensor("wvl", [DIM, HD], F8, kind="ExternalInput")
    woh_d = nc.dram_tensor("woh", [HL * HD, DIM], F8, kind="ExternalInput")
    wol_d = nc.dram_tensor("wol", [HL * HD, DIM], F8, kind="ExternalInput")
    cos_q = nc.dram_tensor("cos_q", [HD, T], BF16, kind="ExternalInput")
    sin_q = nc.dram_tensor("sin_q", [HD, T], BF16, kind="ExternalInput")
    cos_k = nc.dram_tensor("cos_k", [HD, T], BF16, kind="ExternalInput")
    sin_k = nc.dram_tensor("sin_k", [HD, T], BF16, kind="ExternalInput")
    maskT = nc.dram_tensor("maskT", [128, 128], F32, kind="ExternalInput")
    out = nc.dram_tensor("out", [T, DIM], BF16, kind="ExternalOutput")

    Exp = mybir.ActivationFunctionType.Exp

    with TileContext(nc) as tc:
        with tc.tile_pool(name="consts", bufs=1) as cpool, \
             tc.tile_pool(name="hs", bufs=2) as hpool, \
             tc.tile_pool(name="rope_tmp", bufs=2) as rpool, \
             tc.tile_pool(name="work_ps", bufs=5, space=bass.MemorySpace.PSUM) as wpool, \
             tc.tile_pool(name="ot_ps", bufs=2, space=bass.MemorySpace.PSUM) as otpool, \
             tc.tile_pool(name="rs_ps", bufs=1, space=bass.MemorySpace.PSUM) as rspool, \
             tc.tile_pool(name="et", bufs=6) as epool, \
             tc.tile_pool(name="bc_sb", bufs=2) as bcsbpool, \
             tc.tile_pool(name="tao", bufs=2) as taopool, \
             tc.tile_pool(name="recip", bufs=4) as rcpool, \
             tc.tile_pool(name="out_sb", bufs=6) as xsbpool:

            wqh_t = [cpool.tile([128, KT, HD], F8, tag=f"wqh{m}", name=f"wqh{m}")
                     for m in range(HL)]
            wql_t = [cpool.tile([128, KT, HD], F8, tag=f"wql{m}", name=f"wql{m}")
                     for m in range(HL)]
            wkh_t = cpool.tile([128, KT, HD], F8, tag="wkh")
            wkl_t = cpool.tile([128, KT, HD], F8, tag="wkl")
            wvh_t = cpool.tile([128, KT, HD], F8, tag="wvh")
            wvl_t = cpool.tile([128, KT, HD], F8, tag="wvl")
            cq_sb = cpool.tile([128, T], BF16, tag="cq")
            sq_sb = cpool.tile([128, T], BF16, tag="sq")
            ck_sb = cpool.tile([128, T], BF16, tag="ck")
            sk_sb = cpool.tile([128, T], BF16, tag="sk")
            maskT_sb = cpool.tile([128, 128], F32, tag="maskT")
            ones_k = cpool.tile([128, 1], BF16, tag="ones_k")
            qt_rot = cpool.tile([128, HL, T], BF16, tag="qt")
            kt_rot = cpool.tile([128, T], BF16, tag="kt")
            v_sb = cpool.tile([128, TT, HD], BF16, tag="v")
            aoh = cpool.tile([128, HL, T], F8, tag="aoh")
            aol = cpool.tile([128, HL, T], F8, tag="aol")

            wqh_r = wqh_d.ap().rearrange("(kt p) n -> p kt n", p=128)
            wql_r = wql_d.ap().rearrange("(kt p) n -> p kt n", p=128)
            hsh_r = hsh_d.ap().rearrange("(kt p) t -> p kt t", p=128)
            hsl_r = hsl_d.ap().rearrange("(kt p) t -> p kt t", p=128)

            # startup-critical DMA first: tiles the first matmuls touch
            nc.sync.dma_start(wqh_t[0][:, 0:8, :], wqh_r[:, 0:8, 0:HD])
            nc.sync.dma_start(wqh_t[0][:, 8:KT, :], wqh_r[:, 8:KT, 0:HD])
            nc.sync.dma_start(wql_t[0], wql_r[:, :, 0:HD])

            def late_consts():
                nc.sync.dma_start(cq_sb, cos_q.ap())
                nc.sync.dma_start(sq_sb, sin_q.ap())
                for m in range(1, HL):
                    nc.sync.dma_start(wqh_t[m], wqh_r[:, :, m * HD:(m + 1) * HD])
                    nc.sync.dma_start(wql_t[m], wql_r[:, :, m * HD:(m + 1) * HD])
                nc.sync.dma_start(wkh_t, wkh_d.ap().rearrange("(kt p) n -> p kt n", p=128))
                nc.sync.dma_start(wkl_t, wkl_d.ap().rearrange("(kt p) n -> p kt n", p=128))
                nc.sync.dma_start(ck_sb, cos_k.ap())
                nc.sync.dma_start(sk_sb, sin_k.ap())
                nc.sync.dma_start(wvh_t, wvh_d.ap().rearrange("(kt p) n -> p kt n", p=128))
                nc.sync.dma_start(wvl_t, wvl_d.ap().rearrange("(kt p) n -> p kt n", p=128))
                nc.sync.dma_start(maskT_sb, maskT.ap())
                nc.vector.memset(ones_k, 1.0)

            def rope(ps, out_ap, cos_ap, sin_ap):
                """out = ps*cos + halfswap(ps)*sin  (signs baked into sin)."""
                t1 = rpool.tile([128, CH], F32, tag="r1", name="t1")
                t2 = rpool.tile([128, CH], F32, tag="r2", name="t2")
                nc.vector.tensor_mul(t1, ps, cos_ap)
                nc.vector.tensor_mul(t2[0:64, :], ps[64:128, :], sin_ap[0:64, :])
                nc.vector.tensor_mul(t2[64:128, :], ps[0:64, :], sin_ap[64:128, :])
                nc.vector.tensor_add(out_ap, t1, t2)

            def mm3(ps, st_h, st_l, mv_h, mv_l, ncol):
                """3-term fp8 DoubleRow accumulation over all KT k-tiles.

                st_*/mv_* are callables kp -> AP giving the [128,2,ncol]-ish
                slices for k-tile pair kp. Terms: hh, lh, hl.
                """
                for kp in range(KP):
                    nc.tensor.matmul(ps, st_h(kp), mv_h(kp),
                                     start=(kp == 0), stop=False, perf_mode=DR)
                for kp in range(KP):
                    nc.tensor.matmul(ps, st_l(kp), mv_h(kp),
                                     start=False, stop=False, perf_mode=DR)
                for kp in range(KP):
                    nc.tensor.matmul(ps, st_h(kp), mv_l(kp),
                                     start=False, stop=(kp == KP - 1),
                                     perf_mode=DR)

            def emit_chunk(c):
                t0 = c * CH
                hsh_sb = hpool.tile([128, KT, CH], F8, tag="hsh", name="hsh_sb")
                hsl_sb = hpool.tile([128, KT, CH], F8, tag="hsl", name="hsl_sb")
                for g in range(4):
                    nc.sync.dma_start(hsh_sb[:, g * 8:(g + 1) * 8, :],
                                      hsh_r[:, g * 8:(g + 1) * 8, t0:t0 + CH])
                for g in range(4):
                    nc.sync.dma_start(hsl_sb[:, g * 8:(g + 1) * 8, :],
                                      hsl_r[:, g * 8:(g + 1) * 8, t0:t0 + CH])
                for m in range(HL):
                    ps = wpool.tile([128, CH], F32, tag="work", name="ps_q")
                    mm3(ps,
                        lambda kp, m=m: wqh_t[m][:, 2 * kp:2 * kp + 2, :],
                        lambda kp, m=m: wql_t[m][:, 2 * kp:2 * kp + 2, :],
                        lambda kp: hsh_sb[:, 2 * kp:2 * kp + 2, :],
                        lambda kp: hsl_sb[:, 2 * kp:2 * kp + 2, :], CH)
                    if c == 0 and m == 0:
                        late_consts()
                    rope(ps, qt_rot[:, m, t0:t0 + CH],
                         cq_sb[:, t0:t0 + CH], sq_sb[:, t0:t0 + CH])
                ps = wpool.tile([128, CH], F32, tag="work", name="ps_k")
                mm3(ps,
                    lambda kp: wkh_t[:, 2 * kp:2 * kp + 2, :],
                    lambda kp: wkl_t[:, 2 * kp:2 * kp + 2, :],
                    lambda kp: hsh_sb[:, 2 * kp:2 * kp + 2, :],
                    lambda kp: hsl_sb[:, 2 * kp:2 * kp + 2, :], CH)
                rope(ps, kt_rot[:, t0:t0 + CH],
                     ck_sb[:, t0:t0 + CH], sk_sb[:, t0:t0 + CH])
                for vi in range(CH // 128):
                    tt = t0 // 128 + vi
                    ps = wpool.tile([128, HD], F32, tag="work", name="ps_v")
                    v0 = vi * 128
                    mm3(ps,
                        lambda kp: hsh_sb[:, 2 * kp:2 * kp + 2, v0:v0 + 128],
                        lambda kp: hsl_sb[:, 2 * kp:2 * kp + 2, v0:v0 + 128],
                        lambda kp: wvh_t[:, 2 * kp:2 * kp + 2, :],
                        lambda kp: wvl_t[:, 2 * kp:2 * kp + 2, :], 128)
                    nc.scalar.copy(v_sb[:, tt, :], ps)

            # --- attention group machinery (transposed-scores scheme) ---
            pending = [None]

            def epilogue(st):
                rs, ot, h, q0 = st
                recip = rcpool.tile([1, QB], F32, tag="recip", name="recip")
                nc.vector.reciprocal(recip, rs)
                bcs = bcsbpool.tile([128, QB], F32, tag="bcs", name="bcs")
                nc.gpsimd.partition_broadcast(bcs, recip)
                t = taopool.tile([128, QB], F32, tag="tao", name="tao")
                nc.vector.tensor_mul(t, ot, bcs)
                nc.scalar.copy(aoh[:, h, q0:q0 + QB], t)
                nc.vector.tensor_sub(aol[:, h, q0:q0 + QB], t,
                                     aoh[:, h, q0:q0 + QB])

            def emit_group(b, h, qb):
                q0 = b * S + qb * QB
                n_kt = (qb + 1) * (QB // 128)
                rs = rspool.tile([1, QB], F32, tag="rs", name="rs")
                ot = otpool.tile([128, QB], F32, tag="ot", name="ot")
                ets = [None] * n_kt

                def emit_sc(kt):
                    c0 = max(0, kt - qb * (QB // 128)) * 128
                    sc = wpool.tile([128, QB], F32, tag="work", name="sc")
                    nc.tensor.matmul(
                        sc[:, c0:],
                        kt_rot[:, b * S + kt * 128:b * S + (kt + 1) * 128],
                        qt_rot[:, h, q0 + c0:q0 + QB],
                        start=True, stop=True)
                    jd = kt - qb * (QB // 128)
                    if 0 <= jd < QB // 128:
                        nc.vector.tensor_add(sc[:, jd * 128:(jd + 1) * 128],
                                             sc[:, jd * 128:(jd + 1) * 128],
                                             maskT_sb)
                    et = epool.tile([128, QB], BF16, tag="et", name="et")
                    nc.scalar.activation(et[:, c0:], sc[:, c0:], Exp,
                                         bias=0.0, scale=1.0)
                    ets[kt] = (et, c0)

                for w in range(min(4, n_kt)):
                    emit_sc(w)
                for kt in range(n_kt):
                    if kt + 4 < n_kt:
                        emit_sc(kt + 4)
                    et, c0 = ets[kt]
                    nc.tensor.matmul(rs[:, c0:], ones_k, et[:, c0:],
                                     start=(kt == 0), stop=(kt == n_kt - 1))
                    nc.tensor.matmul(ot[:, c0:], v_sb[:, b * (S // 128) + kt, :],
                                     et[:, c0:], start=(kt == 0),
                                     stop=(kt == n_kt - 1))
                    ets[kt] = None
                    if kt == 0 and pending[0] is not None:
                        epilogue(pending[0])
                        pending[0] = None
                pending[0] = (rs, ot, h, q0)

            # --- interleaved emission: each chunk unblocks a set of groups ---
            # chunk c covers tokens [c*512, (c+1)*512) = batch c//2, q-block c%2
            woh_sb = wol_sb = None
            for c in range(NCHUNK):
                emit_chunk(c)
                b, qb = c // 2, c % 2
                for h in range(HL):
                    emit_group(b, h, qb)
                if c == NCHUNK - 1:
                    # wo reuses hs slots (same size); DMA overlaps the
                    # final attention groups
                    woh_sb = hpool.tile([128, HL, DIM], F8, tag="hsh",
                                        name="woh_sb")
                    wol_sb = hpool.tile([128, HL, DIM], F8, tag="hsl",
                                        name="wol_sb")
                    nc.sync.dma_start(
                        woh_sb, woh_d.ap().rearrange("(kh p) n -> p kh n", p=128))
                    nc.sync.dma_start(
                        wol_sb, wol_d.ap().rearrange("(kh p) n -> p kh n", p=128))
            if pending[0] is not None:
                epilogue(pending[0])
                pending[0] = None

            # ---- output projection (row-parallel Wo, fp8 DoubleRow) ----
            for tt in range(TT):
                ts = tt * 128
                for ni, n0 in enumerate(range(0, DIM, 512)):
                    ps = wpool.tile([128, 512], F32, tag="work", name="ps_o")
                    for hp in range(2):
                        nc.tensor.matmul(
                            ps, aoh[:, 2 * hp:2 * hp + 2, ts:ts + 128],
                            woh_sb[:, 2 * hp:2 * hp + 2, n0:n0 + 512],
                            start=(hp == 0), stop=False, perf_mode=DR)
                    for hp in range(2):
                        nc.tensor.matmul(
                            ps, aol[:, 2 * hp:2 * hp + 2, ts:ts + 128],
                            woh_sb[:, 2 * hp:2 * hp + 2, n0:n0 + 512],
                            start=False, stop=False, perf_mode=DR)
                    for hp in range(2):
                        nc.tensor.matmul(
                            ps, aoh[:, 2 * hp:2 * hp + 2, ts:ts + 128],
                            wol_sb[:, 2 * hp:2 * hp + 2, n0:n0 + 512],
                            start=False, stop=(hp == 1), perf_mode=DR)
                    osb = xsbpool.tile([128, 512], BF16, tag="osb", name="osb")
                    if (tt * 8 + ni) % 2 == 0:
                        nc.scalar.copy(osb, ps)
                    else:
                        nc.vector.tensor_copy(osb, ps)
                    nc.sync.dma_start(out.ap()[ts:ts + 128, n0:n0 + 512], osb)
    nc.compile()
    return nc


def _get_nc():
    if "nc" not in _CACHE:
        _CACHE["nc"] = _build()
    return _CACHE["nc"]


def _split8(x: np.ndarray):
    """Split f32 array into (hi, lo) e4m3 pair with hi + lo ~= x."""
    hi = x.astype(E4M3)
    lo = (x - hi.astype(np.float32)).astype(E4M3)
    return hi, lo


def _prep_inputs(inputs) -> list[dict]:
    bf16 = ml_dtypes.bfloat16
    hs = np.asarray(inputs["hidden_states"], dtype=np.float32).reshape(T, DIM)
    hsT = np.ascontiguousarray(hs.T)
    hsh, hsl = _split8(hsT)

    fc = np.asarray(inputs["freqs_cos"], dtype=np.float32).reshape(T, HD // 2).T
    fs = np.asarray(inputs["freqs_sin"], dtype=np.float32).reshape(T, HD // 2).T
    cos2 = np.concatenate([fc, fc], axis=0)            # [128, T]
    sin2 = np.concatenate([-fs, fs], axis=0)           # signed half-rotation
    cos_qv = np.ascontiguousarray(cos2 * (SCALE / S_Q)).astype(bf16)
    sin_qv = np.ascontiguousarray(sin2 * (SCALE / S_Q)).astype(bf16)
    cos_kv = np.ascontiguousarray(cos2 * (1.0 / S_K)).astype(bf16)
    sin_kv = np.ascontiguousarray(sin2 * (1.0 / S_K)).astype(bf16)

    maskT = np.ascontiguousarray(
        np.asarray(inputs["attention_mask"], dtype=np.float32)[0, 0, :128, :128].T)

    perm = np.concatenate([np.arange(0, HD, 2), np.arange(1, HD, 2)])
    Wq = np.asarray(inputs["Wq"], dtype=np.float32)
    Wk = np.asarray(inputs["Wk"], dtype=np.float32)
    Wv = np.asarray(inputs["Wv"], dtype=np.float32)
    Wo = np.asarray(inputs["Wo"], dtype=np.float32)

    in_maps = []
    for c in range(N_CORES):
        wq_c = np.concatenate(
            [Wq[:, (c * HL + h) * HD:(c * HL + h + 1) * HD][:, perm]
             for h in range(HL)], axis=1) * S_Q
        wk_c = Wk[:, c * HD:(c + 1) * HD][:, perm] * S_K
        wv_c = Wv[:, c * HD:(c + 1) * HD] * S_V
        wo_c = Wo[c * HL * HD:(c + 1) * HL * HD, :] * S_O
        wqh, wql = _split8(np.ascontiguousarray(wq_c))
        wkh, wkl = _split8(np.ascontiguousarray(wk_c))
        wvh, wvl = _split8(np.ascontiguousarray(wv_c))
        woh, wol = _split8(np.ascontiguousarray(wo_c))
        in_maps.append({
            "hsh": hsh, "hsl": hsl,
            "wqh": wqh, "wql": wql,
            "wkh": wkh, "wkl": wkl,
            "wvh": wvh, "wvl": wvl,
            "woh": woh, "wol": wol,
            "cos_q": cos_qv, "sin_q": sin_qv,
            "cos_k": cos_kv, "sin_k": sin_kv,
            "maskT": maskT,
        })
    return in_maps


def kernel(**inputs) -> np.ndarray:
    nc = _get_nc()
    in_maps = _prep_inputs(inputs)
    res = bass_utils.run_bass_kernel_spmd(nc, in_maps,
                                          core_ids=list(range(N_CORES)))
    acc = np.zeros((T, DIM), dtype=np.float32)
    for c in range(N_CORES):
        acc += np.asarray(res.results[c]["out"], dtype=np.float32)
    return (acc * (1.0 / (S_V * S_O))).reshape(B, S, DIM)


# revision 25
# speedup vs baseline: 1.2319x; 1.0537x over previous
"""Trainium2 Bass kernel for MllamaTextSdpaAttention (GQA + RoPE + causal SDPA).

Strategy: tensor-parallel over heads across 8 NeuronCores. Core c owns
q-heads [4c, 4c+4) and kv-head c (kv groups intact). Each core computes
hidden @ Wq/Wk/Wv slices, RoPE, causal attention for its heads, and its
row-slice of the Wo matmul, yielding a partial [T, DIM] output (bf16).
The host sums the 8 partials in f32.

Key techniques:
- All four projections (Q/K/V/O) run on the PE in fp8e4m3 DoubleRow mode
  (2 k-tiles of contraction per instruction at 0.5 cycles/column = 4x the
  bf16 FLOP rate). Accuracy is preserved with a 3-term residual split:
  each operand X is split (host-side for inputs/weights, on-device for
  ao) into Xh = fp8(X), Xl = fp8(X - Xh), and W@X ~= Wh@Xh + Wl@Xh +
  Wh@Xl. Net cost: 0.75x the bf16 column count. Weights are pre-scaled
  (x32 Wq/Wk/Wo, x16 Wv) into e4m3's normal range; descales fold into
  the RoPE tables and the host-side gather.
- All inputs are pre-swizzled on the host into exact SBUF layouts so
  every DMA moves >=512-byte contiguous runs (full 360 GB/s; under 512B
  the DMA engines run at half rate).
- Attention stays bf16: transposed scores (scT = K_rot^T.T @ Q_rot^T),
  exp on Act feeds P@V directly, rowsums via ones-vector matmul, 1/rowsum
  applied to the small out^T tile via GpSimd partition-broadcast. RoPE
  as a half-rotation with host-permuted weight columns. Causality at
  128-block granularity.
- Schedule: per chunk, K and V projections run first, then Q heads with
  attention groups interleaved one head behind (group h emits after
  Q_{h+1}), so DVE RoPE latency and chunk-0's DMA-bound startup overlap
  with PE work. The O projection for chunk c-1's tokens is emitted at the
  end of chunk c (ao complete once chunk c's first group fires the
  deferred epilogue), spreading output DMA across the kernel and leaving
  only chunk 3's O-proj after the last attention group.
"""

import numpy as np
import ml_dtypes

import concourse.bacc as bacc
import concourse.bass as bass
import concourse.bass_isa as bass_isa
import concourse.mybir as mybir
from concourse.tile import TileContext
from concourse import bass_utils

BF16 = mybir.dt.bfloat16
F32 = mybir.dt.float32
F8 = mybir.dt.float8e4
E4M3 = ml_dtypes.float8_e4m3

B, S, DIM = 2, 1024, 4096
T = B * S                     # 2048 tokens, batch-major
N_HEADS, N_KV = 32, 8
HD = 128                      # head dim == partition count
N_CORES = 8
HL = N_HEADS // N_CORES       # 4 local q-heads per core
KT = DIM // 128               # 32 feature tiles
KP = KT // 2                  # 16 k-tile PAIRS (DoubleRow)
CH = 512                      # projection token-chunk
NCHUNK = T // CH
QB = 512                      # attention q-block width
TT = T // 128                 # 16 token tiles global
SCALE = 1.0 / float(np.sqrt(HD))
S_Q = 32.0                    # weight pre-scales for fp8 range
S_K = 32.0
S_V = 16.0
S_O = 32.0
DR = mybir.MatmulPerfMode.DoubleRow

_CACHE: dict = {}


def _build():
    nc = bacc.Bacc("TRN2", target_bir_lowering=False, debug=False,
                   enable_asserts=False, dynamic_dma_scratch_size=2048)

    # all tensors pre-swizzled host-side into SBUF layout (partition-major)
    hsh_d = nc.dram_tensor("hsh", [128, NCHUNK, KT, CH], F8, kind="ExternalInput")
    hsl_d = nc.dram_tensor("hsl", [128, NCHUNK, KT, CH], F8, kind="ExternalInput")
    wqh_d = nc.dram_tensor("wqh", [128, HL, KT, HD], F8, kind="ExternalInput")
    wql_d = nc.dram_tensor("wql", [128, HL, KT, HD], F8, kind="ExternalInput")
    wkh_d = nc.dram_tensor("wkh", [128, KT, HD], F8, kind="ExternalInput")
    wkl_d = nc.dram_tensor("wkl", [128, KT, HD], F8, kind="ExternalInput")
    wvh_d = nc.dram_tensor("wvh", [128, KT, HD], F8, kind="ExternalInput")
    wvl_d = nc.dram_tensor("wvl", [128, KT, HD], F8, kind="ExternalInput")
    woh_d = nc.dram_tensor("woh", [128, HL, DIM], F8, kind="ExternalInput")
    wol_d = nc.dram_tensor("wol", [128, HL, DIM], F8, kind="ExternalInput")
    cos_q = nc.dram_tensor("cos_q", [HD, T], BF16, kind="ExternalInput")
    sin_q = nc.dram_tensor("sin_q", [HD, T], BF16, kind="ExternalInput")
    cos_k = nc.dram_tensor("cos_k", [HD, T], BF16, kind="ExternalInput")
    sin_k = nc.dram_tensor("sin_k", [HD, T], BF16, kind="ExternalInput")
    maskT = nc.dram_tensor("maskT", [128, 128], F32, kind="ExternalInput")
    out = nc.dram_tensor("out", [T, DIM], BF16, kind="ExternalOutput")

    Exp = mybir.ActivationFunctionType.Exp

    with TileContext(nc) as tc:
        with tc.tile_pool(name="consts", bufs=1) as cpool, \
             tc.tile_pool(name="hs", bufs=2) as hpool, \
             tc.tile_pool(name="rope_tmp", bufs=1) as rpool, \
             tc.tile_pool(name="work_ps", bufs=6, space=bass.MemorySpace.PSUM) as wpool, \
             tc.tile_pool(name="ot_ps", bufs=2, space=bass.MemorySpace.PSUM) as otpool, \
             tc.tile_pool(name="et", bufs=5) as epool, \
             tc.tile_pool(name="esum", bufs=2) as espool, \
             tc.tile_pool(name="tao", bufs=2) as taopool, \
             tc.tile_pool(name="out_sb", bufs=4) as xsbpool:

            wqh_t = [cpool.tile([128, KT, HD], F8, tag=f"wqh{m}", name=f"wqh{m}")
                     for m in range(HL)]
            wql_t = [cpool.tile([128, KT, HD], F8, tag=f"wql{m}", name=f"wql{m}")
                     for m in range(HL)]
            wkh_t = cpool.tile([128, KT, HD], F8, tag="wkh")
            wkl_t = cpool.tile([128, KT, HD], F8, tag="wkl")
            wvh_t = cpool.tile([128, KT, HD], F8, tag="wvh")
            wvl_t = cpool.tile([128, KT, HD], F8, tag="wvl")
            woh_sb = cpool.tile([128, HL, DIM], F8, tag="woh")
            wol_sb = cpool.tile([128, HL, DIM], F8, tag="wol")
            cq_sb = cpool.tile([128, T], BF16, tag="cq")
            sq_sb = cpool.tile([128, T], BF16, tag="sq")
            ck_sb = cpool.tile([128, T], BF16, tag="ck")
            sk_sb = cpool.tile([128, T], BF16, tag="sk")
            maskT_sb = cpool.tile([128, 128], F32, tag="maskT")
            qt_rot = cpool.tile([128, HL, T], BF16, tag="qt")
            kt_rot = cpool.tile([128, T], BF16, tag="kt")
            v_sb = cpool.tile([128, TT, HD], BF16, tag="v")
            aoh = cpool.tile([128, HL, T], F8, tag="aoh")
            aol = cpool.tile([128, HL, T], F8, tag="aol")

            # startup-critical DMA first: K-projection weights
            nc.sync.dma_start(wkh_t[:, 0:8, :], wkh_d.ap()[:, 0:8, :])
            nc.sync.dma_start(wkh_t[:, 8:KT, :], wkh_d.ap()[:, 8:KT, :])
            nc.sync.dma_start(wkl_t, wkl_d.ap())

            def emit_hs_dmas(c):
                hsh_sb = hpool.tile([128, KT, CH], F8, tag="hsh", name="hsh_sb")
                hsl_sb = hpool.tile([128, KT, CH], F8, tag="hsl", name="hsl_sb")
                for g in range(4):
                    nc.sync.dma_start(hsh_sb[:, g * 8:(g + 1) * 8, :],
                                      hsh_d.ap()[:, c, g * 8:(g + 1) * 8, :])
                for g in range(4):
                    nc.sync.dma_start(hsl_sb[:, g * 8:(g + 1) * 8, :],
                                      hsl_d.ap()[:, c, g * 8:(g + 1) * 8, :])
                return hsh_sb, hsl_sb

            def late_consts():
                # ordered by first use: K rope, V proj, Q0 weights+rope, ...
                nc.sync.dma_start(ck_sb, cos_k.ap())
                nc.sync.dma_start(sk_sb, sin_k.ap())
                nc.sync.dma_start(wvh_t, wvh_d.ap())
                nc.sync.dma_start(wvl_t, wvl_d.ap())
                nc.sync.dma_start(wqh_t[0], wqh_d.ap()[:, 0])
                nc.sync.dma_start(wql_t[0], wql_d.ap()[:, 0])
                nc.sync.dma_start(cq_sb, cos_q.ap())
                nc.sync.dma_start(sq_sb, sin_q.ap())
                nc.sync.dma_start(wqh_t[1], wqh_d.ap()[:, 1])
                nc.sync.dma_start(wql_t[1], wql_d.ap()[:, 1])
                nc.sync.dma_start(maskT_sb, maskT.ap())
                for m in range(2, HL):
                    nc.sync.dma_start(wqh_t[m], wqh_d.ap()[:, m])
                    nc.sync.dma_start(wql_t[m], wql_d.ap()[:, m])

            def rope(ps, out_ap, cos_ap, sin_ap):
                """out = ps*cos + halfswap(ps)*sin  (signs baked into sin)."""
                t1 = rpool.tile([128, CH], F32, tag="r1", name="t1")
                t2 = rpool.tile([128, CH], F32, tag="r2", name="t2")
                nc.vector.tensor_mul(t1, ps, cos_ap)
                nc.vector.tensor_mul(t2[0:64, :], ps[64:128, :], sin_ap[0:64, :])
                nc.vector.tensor_mul(t2[64:128, :], ps[0:64, :], sin_ap[64:128, :])
                nc.vector.tensor_add(out_ap, t1, t2)

            def mm3(ps, st_h, st_l, mv_h, mv_l, hook=None):
                """3-term fp8 DoubleRow accumulation over all KT k-tiles."""
                for kp in range(KP):
                    nc.tensor.matmul(ps, st_h(kp), mv_h(kp),
                                     start=(kp == 0), stop=False, perf_mode=DR)
                if hook is not None:
                    hook()
                for kp in range(KP):
                    nc.tensor.matmul(ps, st_l(kp), mv_h(kp),
                                     start=False, stop=False, perf_mode=DR)
                for kp in range(KP):
                    nc.tensor.matmul(ps, st_h(kp), mv_l(kp),
                                     start=False, stop=(kp == KP - 1),
                                     perf_mode=DR)

            # --- attention group machinery (transposed-scores scheme) ---
            pending = [None]

            def epilogue(st):
                rs, ot, h, q0 = st
                with nc.allow_low_precision("softmax rowsum recip in bf16"):
                    nc.vector.reciprocal(rs, rs)
                t = taopool.tile([128, QB], F32, tag="tao", name="tao")
                nc.vector.tensor_mul(t, ot, rs)
                nc.scalar.copy(aoh[:, h, q0:q0 + QB], t)
                nc.vector.tensor_sub(aol[:, h, q0:q0 + QB], t,
                                     aoh[:, h, q0:q0 + QB])

            def group_units(b, h, qb):
                """Generator: one yield per consumed score k-tile, so group
                work (Act-heavy exp) can be interleaved into PE-heavy Q/O
                projection streams."""
                q0 = b * S + qb * QB
                n_kt = (qb + 1) * (QB // 128)
                # esum accumulates sum_kt et_kt elementwise on DVE (bf16, 2x
                # mode); the final GpSimd partition_all_reduce turns it into
                # softmax rowsums broadcast across partitions. Keeps the
                # rowsum off the PE; bf16 accumulation costs ~0.5% on rs,
                # well inside the error budget.
                esum = espool.tile([128, QB], BF16, tag="esum", name="esum")
                ot = otpool.tile([128, QB], F32, tag="ot", name="ot")
                ets = [None] * n_kt

                def emit_sc(kt):
                    c0 = max(0, kt - qb * (QB // 128)) * 128
                    sc = wpool.tile([128, QB], F32, tag="work", name="sc")
                    nc.tensor.matmul(
                        sc[:, c0:],
                        kt_rot[:, b * S + kt * 128:b * S + (kt + 1) * 128],
                        qt_rot[:, h, q0 + c0:q0 + QB],
                        start=True, stop=True)
                    jd = kt - qb * (QB // 128)
                    if 0 <= jd < QB // 128:
                        nc.vector.tensor_add(sc[:, jd * 128:(jd + 1) * 128],
                                             sc[:, jd * 128:(jd + 1) * 128],
                                             maskT_sb)
                    et = epool.tile([128, QB], BF16, tag="et", name="et")
                    nc.scalar.activation(et[:, c0:], sc[:, c0:], Exp,
                                         bias=0.0, scale=1.0)
                    ets[kt] = (et, c0)

                def consume(kt):
                    et, c0 = ets[kt]
                    if kt == 0:
                        nc.vector.tensor_copy(esum, et)
                    else:
                        nc.vector.tensor_add(esum[:, c0:], esum[:, c0:],
                                             et[:, c0:])
                    nc.tensor.matmul(ot[:, c0:], v_sb[:, b * (S // 128) + kt, :],
                                     et[:, c0:], start=(kt == 0),
                                     stop=(kt == n_kt - 1))
                    ets[kt] = None
                    if kt == 0 and pending[0] is not None:
                        epilogue(pending[0])
                        pending[0] = None

                for kt in range(n_kt):
                    emit_sc(kt)
                    if kt >= 2:
                        consume(kt - 2)
                        yield
                for kt in range(max(0, n_kt - 2), n_kt):
                    consume(kt)
                    yield
                nc.gpsimd.partition_all_reduce(esum, esum, 128,
                                               bass_isa.ReduceOp.add)
                pending[0] = (esum, ot, h, q0)

            def oproj_units(c):
                """Generator: one yield per O-projection psum tile (fp8
                DoubleRow) for chunk c's 4 token tiles."""
                for tt in range(c * 4, c * 4 + 4):
                    ts = tt * 128
                    for n0 in range(0, DIM, 512):
                        ps = wpool.tile([128, 512], F32, tag="work", name="ps_o")
                        for hp in range(2):
                            nc.tensor.matmul(
                                ps, aoh[:, 2 * hp:2 * hp + 2, ts:ts + 128],
                                woh_sb[:, 2 * hp:2 * hp + 2, n0:n0 + 512],
                                start=(hp == 0), stop=False, perf_mode=DR)
                        for hp in range(2):
                            nc.tensor.matmul(
                                ps, aol[:, 2 * hp:2 * hp + 2, ts:ts + 128],
                                woh_sb[:, 2 * hp:2 * hp + 2, n0:n0 + 512],
                                start=False, stop=False, perf_mode=DR)
                        for hp in range(2):
                            nc.tensor.matmul(
                                ps, aoh[:, 2 * hp:2 * hp + 2, ts:ts + 128],
                                wol_sb[:, 2 * hp:2 * hp + 2, n0:n0 + 512],
                                start=False, stop=(hp == 1), perf_mode=DR)
                        osb = xsbpool.tile([128, 512], BF16, tag="osb",
                                           name="osb")
                        if (tt * 8 + n0 // 512) % 2 == 0:
                            nc.scalar.copy(osb, ps)
                        else:
                            nc.vector.tensor_copy(osb, ps)
                        nc.sync.dma_start(out.ap()[ts:ts + 128, n0:n0 + 512],
                                          osb)
                        yield

            def drain(gen, n=10 ** 9):
                """Pull up to n units; True if the generator is exhausted."""
                for _ in range(n):
                    if next(gen, _SENTINEL) is _SENTINEL:
                        return True
                return False

            _SENTINEL = object()

            def hs_dma_closures(c):
                """Allocate next chunk's hs tiles; return deferred DMA
                emitters so the transfers can be paced into the O-proj
                stream (fair-sharing the DMA engines with osb writes)."""
                hsh_sb = hpool.tile([128, KT, CH], F8, tag="hsh", name="hsh_sb")
                hsl_sb = hpool.tile([128, KT, CH], F8, tag="hsl", name="hsl_sb")

                def mk(dst, src, g):
                    return lambda: nc.sync.dma_start(
                        dst[:, g * 8:(g + 1) * 8, :],
                        src[:, c, g * 8:(g + 1) * 8, :])

                fs = [mk(hsh_sb, hsh_d.ap(), g) for g in range(4)]
                fs += [mk(hsl_sb, hsl_d.ap(), g) for g in range(4)]
                return (hsh_sb, hsl_sb), fs

            # --- main schedule ---
            hs_cur = emit_hs_dmas(0)
            for c in range(NCHUNK):
                hsh_sb, hsl_sb = hs_cur
                t0 = c * CH
                b, qb = c // 2, c % 2
                per_batch = 2 if qb else 1   # group units per Q quarter-batch

                def st(w):
                    return lambda kp: w[:, 2 * kp:2 * kp + 2, :]

                def mv(x):
                    return lambda kp: x[:, 2 * kp:2 * kp + 2, :]

                # K projection first (its rope unblocks all groups)
                ps = wpool.tile([128, CH], F32, tag="work", name="ps_k")
                mm3(ps, st(wkh_t), st(wkl_t), mv(hsh_sb), mv(hsl_sb),
                    hook=(late_consts if c == 0 else None))
                rope(ps, kt_rot[:, t0:t0 + CH],
                     ck_sb[:, t0:t0 + CH], sk_sb[:, t0:t0 + CH])
                # V projection
                for vi in range(CH // 128):
                    tt = t0 // 128 + vi
                    ps = wpool.tile([128, HD], F32, tag="work", name="ps_v")
                    v0 = vi * 128
                    mm3(ps,
                        lambda kp: hsh_sb[:, 2 * kp:2 * kp + 2, v0:v0 + 128],
                        lambda kp: hsl_sb[:, 2 * kp:2 * kp + 2, v0:v0 + 128],
                        st(wvh_t), st(wvl_t))
                    nc.scalar.copy(v_sb[:, tt, :], ps)
                # Q heads in quarter-batches; group h-2's units interleave
                # into head h's matmul stream (PE-heavy, Act-light)
                active = None
                for m in range(HL):
                    ps = wpool.tile([128, CH], F32, tag="work", name="ps_q")
                    sh, sl = st(wqh_t[m]), st(wql_t[m])
                    mh, ml = mv(hsh_sb), mv(hsl_sb)
                    for bi in range(4):
                        k0, k1 = bi * 4, bi * 4 + 4
                        for kp in range(k0, k1):
                            nc.tensor.matmul(ps, sh(kp), mh(kp),
                                             start=(kp == 0), stop=False,
                                             perf_mode=DR)
                        for kp in range(k0, k1):
                            nc.tensor.matmul(ps, sl(kp), mh(kp),
                                             start=False, stop=False,
                                             perf_mode=DR)
                        for kp in range(k0, k1):
                            nc.tensor.matmul(ps, sh(kp), ml(kp), start=False,
                                             stop=(bi == 3 and kp == k1 - 1),
                                             perf_mode=DR)
                        if active is not None:
                            drain(active, per_batch)
                    rope(ps, qt_rot[:, m, t0:t0 + CH],
                         cq_sb[:, t0:t0 + CH], sq_sb[:, t0:t0 + CH])
                    if m >= 1:
                        if active is not None:
                            drain(active)
                        active = group_units(b, m - 1, qb)
                # remaining groups (h=2,3) interleave with the O projection
                # of the previous chunk; chunk 0 has no O-proj to interleave.
                # Next-chunk hs DMAs (and wo, at c==0) pace into the stream.
                tail_gens = [active, group_units(b, HL - 1, qb)]
                active = None
                feed = []
                if c + 1 < NCHUNK:
                    hs_cur, feed = hs_dma_closures(c + 1)
                if c == 0:
                    feed.append(lambda: nc.sync.dma_start(woh_sb, woh_d.ap()))
                    feed.append(lambda: nc.sync.dma_start(wol_sb, wol_d.ap()))
                if c >= 1:
                    op = oproj_units(c - 1)
                    gi = 0
                    done_op = False
                    opn = 0
                    while not done_op:
                        if gi < len(tail_gens):
                            if drain(tail_gens[gi], 1):
                                gi += 1
                                continue
                        done_op = drain(op, 2 if qb else 4)
                        opn += 1
                        if feed and opn % 2 == 0:
                            feed.pop(0)()
                    for g in tail_gens[gi:]:
                        drain(g)
                else:
                    for f in feed:
                        f()
                    feed = []
                    for g in tail_gens:
                        drain(g)
                for f in feed:
                    f()
                if c == NCHUNK - 1 and pending[0] is not None:
                    # flush the last epilogue now: its DVE/Act ops run while
                    # the PE works through the final O projection below
                    epilogue(pending[0])
                    pending[0] = None
            drain(oproj_units(NCHUNK - 1))
    nc.compile()
    return nc


def _get_nc():
    if "nc" not in _CACHE:
        _CACHE["nc"] = _build()
    return _CACHE["nc"]


def _split8(x: np.ndarray):
    """Split f32 array into (hi, lo) e4m3 pair with hi + lo ~= x."""
    hi = x.astype(E4M3)
    lo = (x - hi.astype(np.float32)).astype(E4M3)
    return hi, lo


def _prep_inputs(inputs) -> list[dict]:
    bf16 = ml_dtypes.bfloat16
    hs = np.asarray(inputs["hidden_states"], dtype=np.float32).reshape(T, DIM)
    hsT = np.ascontiguousarray(hs.T)
    hsh, hsl = _split8(hsT)

    def swz_hs(x):  # [DIM, T] -> [128, NCHUNK, KT, CH] (SBUF layout)
        return np.ascontiguousarray(
            x.reshape(KT, 128, NCHUNK, CH).transpose(1, 2, 0, 3))

    hsh = swz_hs(hsh)
    hsl = swz_hs(hsl)

    fc = np.asarray(inputs["freqs_cos"], dtype=np.float32).reshape(T, HD // 2).T
    fs = np.asarray(inputs["freqs_sin"], dtype=np.float32).reshape(T, HD // 2).T
    cos2 = np.concatenate([fc, fc], axis=0)            # [128, T]
    sin2 = np.concatenate([-fs, fs], axis=0)           # signed half-rotation
    cos_qv = np.ascontiguousarray(cos2 * (SCALE / S_Q)).astype(bf16)
    sin_qv = np.ascontiguousarray(sin2 * (SCALE / S_Q)).astype(bf16)
    cos_kv = np.ascontiguousarray(cos2 * (1.0 / S_K)).astype(bf16)
    sin_kv = np.ascontiguousarray(sin2 * (1.0 / S_K)).astype(bf16)

    maskT = np.ascontiguousarray(
        np.asarray(inputs["attention_mask"], dtype=np.float32)[0, 0, :128, :128].T)

    perm = np.concatenate([np.arange(0, HD, 2), np.arange(1, HD, 2)])
    Wq = np.asarray(inputs["Wq"], dtype=np.float32)
    Wk = np.asarray(inputs["Wk"], dtype=np.float32)
    Wv = np.asarray(inputs["Wv"], dtype=np.float32)
    Wo = np.asarray(inputs["Wo"], dtype=np.float32)

    def swz_w(x, nh):  # [DIM, nh*HD] -> [128, nh, KT, HD]
        return np.ascontiguousarray(
            x.reshape(KT, 128, nh, HD).transpose(1, 2, 0, 3))

    in_maps = []
    for c in range(N_CORES):
        wq_c = np.concatenate(
            [Wq[:, (c * HL + h) * HD:(c * HL + h + 1) * HD][:, perm]
             for h in range(HL)], axis=1) * S_Q
        wk_c = Wk[:, c * HD:(c + 1) * HD][:, perm] * S_K
        wv_c = Wv[:, c * HD:(c + 1) * HD] * S_V
        wo_c = Wo[c * HL * HD:(c + 1) * HL * HD, :] * S_O
        wqh, wql = _split8(wq_c)
        wkh, wkl = _split8(wk_c)
        wvh, wvl = _split8(wv_c)
        woh, wol = _split8(wo_c)
        in_maps.append({
            "hsh": hsh, "hsl": hsl,
            "wqh": swz_w(wqh, HL), "wql": swz_w(wql, HL),
            "wkh": swz_w(wkh, 1).reshape(128, KT, HD),
            "wkl": swz_w(wkl, 1).reshape(128, KT, HD),
            "wvh": swz_w(wvh, 1).reshape(128, KT, HD),
            "wvl": swz_w(wvl, 1).reshape(128, KT, HD),
            "woh": np.ascontiguousarray(
                woh.reshape(HL, 128, DIM).transpose(1, 0, 2)),
            "wol": np.ascontiguousarray(
                wol.reshape(HL, 128, DIM).transpose(1, 0, 2)),
            "cos_q": cos_qv, "sin_q": sin_qv,
            "cos_k": cos_kv, "sin_k": sin_kv,
            "maskT": maskT,
        })
    return in_maps


def kernel(**inputs) -> np.ndarray:
    nc = _get_nc()
    in_maps = _prep_inputs(inputs)
    res = bass_utils.run_bass_kernel_spmd(nc, in_maps,
                                          core_ids=list(range(N_CORES)))
    acc = np.zeros((T, DIM), dtype=np.float32)
    for c in range(N_CORES):
        acc += np.asarray(res.results[c]["out"], dtype=np.float32)
    return (acc * (1.0 / (S_V * S_O))).reshape(B, S, DIM)


# revision 45
# speedup vs baseline: 1.2996x; 1.0549x over previous
"""Trainium2 Bass kernel for MllamaTextSdpaAttention (GQA + RoPE + causal SDPA).

Strategy: tensor-parallel over heads across 8 NeuronCores. Core c owns
q-heads [4c, 4c+4) and kv-head c (kv groups intact). Each core computes
hidden @ Wq/Wk/Wv slices, RoPE, causal attention for its heads, and its
row-slice of the Wo matmul, yielding a partial [T, DIM] output (bf16).
The host sums the 8 partials in f32.

Key techniques:
- All four projections (Q/K/V/O) run on the PE in fp8e4m3 DoubleRow mode
  (2 k-tiles of contraction per instruction at 0.5 cycles/column = 4x the
  bf16 FLOP rate). Accuracy is preserved with a 3-term residual split:
  each operand X is split (host-side for inputs/weights, on-device for
  ao) into Xh = fp8(X), Xl = fp8(X - Xh), and W@X ~= Wh@Xh + Wl@Xh +
  Wh@Xl. Net cost: 0.75x the bf16 column count. Weights are pre-scaled
  (x32 Wq/Wk/Wo, x16 Wv) into e4m3's normal range; descales fold into
  the RoPE tables and the host-side gather.
- All inputs are pre-swizzled on the host into exact SBUF layouts so
  every DMA moves >=512-byte contiguous runs (full 360 GB/s; under 512B
  the DMA engines run at half rate).
- Attention stays bf16: transposed scores (scT = K_rot^T.T @ Q_rot^T),
  exp on Act feeds P@V directly, rowsums via ones-vector matmul, 1/rowsum
  applied to the small out^T tile via GpSimd partition-broadcast. RoPE
  as a half-rotation with host-permuted weight columns. Causality at
  128-block granularity.
- Schedule: per chunk, K and V projections run first, then Q heads with
  attention groups interleaved one head behind (group h emits after
  Q_{h+1}), so DVE RoPE latency and chunk-0's DMA-bound startup overlap
  with PE work. The O projection for chunk c-1's tokens is emitted at the
  end of chunk c (ao complete once chunk c's first group fires the
  deferred epilogue), spreading output DMA across the kernel and leaving
  only chunk 3's O-proj after the last attention group.
"""

import numpy as np
import ml_dtypes

import concourse.bacc as bacc
import concourse.bass as bass
import concourse.bass_isa as bass_isa
import concourse.mybir as mybir
from concourse.tile import TileContext
from concourse import bass_utils

BF16 = mybir.dt.bfloat16
F32 = mybir.dt.float32
F8 = mybir.dt.float8e4
E4M3 = ml_dtypes.float8_e4m3

B, S, DIM = 2, 1024, 4096
T = B * S                     # 2048 tokens, batch-major
N_HEADS, N_KV = 32, 8
HD = 128                      # head dim == partition count
N_CORES = 8
HL = N_HEADS // N_CORES       # 4 local q-heads per core
KT = DIM // 128               # 32 feature tiles
KP = KT // 2                  # 16 k-tile PAIRS (DoubleRow)
CH = 512                      # projection token-chunk
NCHUNK = T // CH
QB = 512                      # attention q-block width
TT = T // 128                 # 16 token tiles global
SCALE = 1.0 / float(np.sqrt(HD))
S_Q = 32.0                    # weight pre-scales for fp8 range
S_K = 32.0
S_V = 16.0
S_O = 32.0
DR = mybir.MatmulPerfMode.DoubleRow

_CACHE: dict = {}


def _build():
    nc = bacc.Bacc("TRN2", target_bir_lowering=False, debug=False,
                   enable_asserts=False, dynamic_dma_scratch_size=2048)

    # all tensors pre-swizzled host-side into SBUF layout (partition-major)
    hsh_d = nc.dram_tensor("hsh", [128, NCHUNK, KT, CH], F8, kind="ExternalInput")
    hsl_d = nc.dram_tensor("hsl", [128, NCHUNK, KT, CH], F8, kind="ExternalInput")
    wqh_d = nc.dram_tensor("wqh", [128, HL, KT, HD], F8, kind="ExternalInput")
    wql_d = nc.dram_tensor("wql", [128, HL, KT, HD], F8, kind="ExternalInput")
    wkh_d = nc.dram_tensor("wkh", [128, KT, HD], F8, kind="ExternalInput")
    wkl_d = nc.dram_tensor("wkl", [128, KT, HD], F8, kind="ExternalInput")
    wvh_d = nc.dram_tensor("wvh", [128, KT, HD], F8, kind="ExternalInput")
    wvl_d = nc.dram_tensor("wvl", [128, KT, HD], F8, kind="ExternalInput")
    woh_d = nc.dram_tensor("woh", [128, HL, DIM], F8, kind="ExternalInput")
    wol_d = nc.dram_tensor("wol", [128, HL, DIM], F8, kind="ExternalInput")
    cos_q = nc.dram_tensor("cos_q", [HD, T], BF16, kind="ExternalInput")
    sin_q = nc.dram_tensor("sin_q", [HD, T], BF16, kind="ExternalInput")
    cos_k = nc.dram_tensor("cos_k", [HD, T], BF16, kind="ExternalInput")
    sin_k = nc.dram_tensor("sin_k", [HD, T], BF16, kind="ExternalInput")
    maskT = nc.dram_tensor("maskT", [128, 128], F32, kind="ExternalInput")
    out = nc.dram_tensor("out", [T, DIM], BF16, kind="ExternalOutput")

    Exp = mybir.ActivationFunctionType.Exp

    with TileContext(nc) as tc:
        with tc.tile_pool(name="consts", bufs=1) as cpool, \
             tc.tile_pool(name="hs", bufs=2) as hpool, \
             tc.tile_pool(name="rope_tmp", bufs=1) as rpool, \
             tc.tile_pool(name="work_ps", bufs=6, space=bass.MemorySpace.PSUM) as wpool, \
             tc.tile_pool(name="ot_ps", bufs=2, space=bass.MemorySpace.PSUM) as otpool, \
             tc.tile_pool(name="et", bufs=5) as epool, \
             tc.tile_pool(name="esum", bufs=2) as espool, \
             tc.tile_pool(name="tao", bufs=1) as taopool, \
             tc.tile_pool(name="out_sb", bufs=6) as xsbpool:

            wqh_t = [cpool.tile([128, KT, HD], F8, tag=f"wqh{m}", name=f"wqh{m}")
                     for m in range(HL)]
            wql_t = [cpool.tile([128, KT, HD], F8, tag=f"wql{m}", name=f"wql{m}")
                     for m in range(HL)]
            wkh_t = cpool.tile([128, KT, HD], F8, tag="wkh")
            wkl_t = cpool.tile([128, KT, HD], F8, tag="wkl")
            wvh_t = cpool.tile([128, KT, HD], F8, tag="wvh")
            wvl_t = cpool.tile([128, KT, HD], F8, tag="wvl")
            woh_sb = cpool.tile([128, HL, DIM], F8, tag="woh")
            wol_sb = cpool.tile([128, HL, DIM], F8, tag="wol")
            cq_sb = cpool.tile([128, T], BF16, tag="cq")
            sq_sb = cpool.tile([128, T], BF16, tag="sq")
            ck_sb = cpool.tile([128, T], BF16, tag="ck")
            sk_sb = cpool.tile([128, T], BF16, tag="sk")
            maskT_sb = cpool.tile([128, 128], F32, tag="maskT")
            qt_rot = cpool.tile([128, HL, T], BF16, tag="qt")
            kt_rot = cpool.tile([128, T], BF16, tag="kt")
            v_sb = cpool.tile([128, TT, HD], BF16, tag="v")
            aoh = cpool.tile([128, HL, T], F8, tag="aoh")
            aol = cpool.tile([128, HL, T], F8, tag="aol")

            # startup-critical DMA first: K-projection weights
            nc.sync.dma_start(wkh_t[:, 0:8, :], wkh_d.ap()[:, 0:8, :])
            nc.sync.dma_start(wkh_t[:, 8:KT, :], wkh_d.ap()[:, 8:KT, :])
            nc.sync.dma_start(wkl_t, wkl_d.ap())

            def emit_hs_dmas(c, lo=True):
                hsh_sb = hpool.tile([128, KT, CH], F8, tag="hsh", name="hsh_sb")
                hsl_sb = hpool.tile([128, KT, CH], F8, tag="hsl", name="hsl_sb")
                for g in range(4):
                    nc.sync.dma_start(hsh_sb[:, g * 8:(g + 1) * 8, :],
                                      hsh_d.ap()[:, c, g * 8:(g + 1) * 8, :])
                if lo:
                    for g in range(4):
                        nc.sync.dma_start(hsl_sb[:, g * 8:(g + 1) * 8, :],
                                          hsl_d.ap()[:, c, g * 8:(g + 1) * 8, :])
                return hsh_sb, hsl_sb

            def late_consts(hsl_sb):
                # strictly ordered by first use under the term-staged chunk-0
                # emission: V terms, then K-hl/V-lh (hsl), then Q0, ropes, Q1+
                nc.sync.dma_start(wvh_t, wvh_d.ap())
                nc.sync.dma_start(wvl_t, wvl_d.ap())
                for g in range(4):
                    nc.sync.dma_start(hsl_sb[:, g * 8:(g + 1) * 8, :],
                                      hsl_d.ap()[:, 0, g * 8:(g + 1) * 8, :])
                nc.sync.dma_start(cq_sb, cos_q.ap())
                nc.sync.dma_start(sq_sb, sin_q.ap())
                nc.sync.dma_start(maskT_sb, maskT.ap())
                nc.sync.dma_start(wqh_t[0], wqh_d.ap()[:, 0])
                nc.sync.dma_start(wql_t[0], wql_d.ap()[:, 0])
                nc.sync.dma_start(ck_sb, cos_k.ap())
                nc.sync.dma_start(sk_sb, sin_k.ap())
                nc.sync.dma_start(wqh_t[1], wqh_d.ap()[:, 1])
                nc.sync.dma_start(wql_t[1], wql_d.ap()[:, 1])
                for m in range(2, HL):
                    nc.sync.dma_start(wqh_t[m], wqh_d.ap()[:, m])
                    nc.sync.dma_start(wql_t[m], wql_d.ap()[:, m])

            def rope(ps, out_ap, cos_ap, sin_ap):
                """out = ps*cos + halfswap(ps)*sin  (signs baked into sin)."""
                t1 = rpool.tile([128, CH], F32, tag="r1", name="t1")
                t2 = rpool.tile([128, CH], F32, tag="r2", name="t2")
                nc.vector.tensor_mul(t1, ps, cos_ap)
                nc.vector.tensor_mul(t2[0:64, :], ps[64:128, :], sin_ap[0:64, :])
                nc.vector.tensor_mul(t2[64:128, :], ps[0:64, :], sin_ap[64:128, :])
                nc.vector.tensor_add(out_ap, t1, t2)

            def mm3(ps, st_h, st_l, mv_h, mv_l):
                """3-term fp8 DoubleRow accumulation over all KT k-tiles."""
                for kp in range(KP):
                    nc.tensor.matmul(ps, st_h(kp), mv_h(kp),
                                     start=(kp == 0), stop=False, perf_mode=DR)
                for kp in range(KP):
                    nc.tensor.matmul(ps, st_l(kp), mv_h(kp),
                                     start=False, stop=False, perf_mode=DR)
                for kp in range(KP):
                    nc.tensor.matmul(ps, st_h(kp), mv_l(kp),
                                     start=False, stop=(kp == KP - 1),
                                     perf_mode=DR)

            # --- attention group machinery (transposed-scores scheme) ---
            pending = [None]

            def epilogue(st):
                rs, ot, h, q0 = st
                with nc.allow_low_precision("softmax rowsum recip in bf16"):
                    nc.vector.reciprocal(rs, rs)
                t = taopool.tile([128, QB], F32, tag="tao", name="tao")
                nc.vector.tensor_mul(t, ot, rs)
                nc.scalar.copy(aoh[:, h, q0:q0 + QB], t)
                nc.vector.tensor_sub(aol[:, h, q0:q0 + QB], t,
                                     aoh[:, h, q0:q0 + QB])

            def group_units(b, h, qb):
                """Generator: one yield per consumed score k-tile, so group
                work (Act-heavy exp) can be interleaved into PE-heavy Q/O
                projection streams."""
                q0 = b * S + qb * QB
                n_kt = (qb + 1) * (QB // 128)
                # esum accumulates sum_kt et_kt elementwise on DVE (bf16, 2x
                # mode); the final GpSimd partition_all_reduce turns it into
                # softmax rowsums broadcast across partitions. Keeps the
                # rowsum off the PE; bf16 accumulation costs ~0.5% on rs,
                # well inside the error budget.
                esum = espool.tile([128, QB], BF16, tag="esum", name="esum")
                ot = otpool.tile([128, QB], F32, tag="ot", name="ot")
                ets = [None] * n_kt

                def emit_sc(kt):
                    c0 = max(0, kt - qb * (QB // 128)) * 128
                    sc = wpool.tile([128, QB], F32, tag="work", name="sc")
                    nc.tensor.matmul(
                        sc[:, c0:],
                        kt_rot[:, b * S + kt * 128:b * S + (kt + 1) * 128],
                        qt_rot[:, h, q0 + c0:q0 + QB],
                        start=True, stop=True)
                    jd = kt - qb * (QB // 128)
                    if 0 <= jd < QB // 128:
                        nc.vector.tensor_add(sc[:, jd * 128:(jd + 1) * 128],
                                             sc[:, jd * 128:(jd + 1) * 128],
                                             maskT_sb)
                    et = epool.tile([128, QB], BF16, tag="et", name="et")
                    nc.scalar.activation(et[:, c0:], sc[:, c0:], Exp,
                                         bias=0.0, scale=1.0)
                    ets[kt] = (et, c0)

                def consume(kt):
                    et, c0 = ets[kt]
                    if kt == 0:
                        nc.vector.tensor_copy(esum, et)
                    else:
                        nc.vector.tensor_add(esum[:, c0:], esum[:, c0:],
                                             et[:, c0:])
                    nc.tensor.matmul(ot[:, c0:], v_sb[:, b * (S // 128) + kt, :],
                                     et[:, c0:], start=(kt == 0),
                                     stop=(kt == n_kt - 1))
                    ets[kt] = None
                    if kt == 0 and pending[0] is not None:
                        epilogue(pending[0])
                        pending[0] = None

                for kt in range(n_kt):
                    emit_sc(kt)
                    if kt >= 2:
                        consume(kt - 2)
                        yield
                for kt in range(max(0, n_kt - 2), n_kt):
                    consume(kt)
                    yield
                nc.gpsimd.partition_all_reduce(esum, esum, 128,
                                               bass_isa.ReduceOp.add)
                pending[0] = (esum, ot, h, q0)

            def oproj_units(c, split_copies=False):
                """Generator: one yield per O-projection psum tile (fp8
                DoubleRow) for chunk c's 4 token tiles."""
                for tt in range(c * 4, c * 4 + 4):
                    ts = tt * 128
                    for n0 in range(0, DIM, 512):
                        ps = wpool.tile([128, 512], F32, tag="work", name="ps_o")
                        for hp in range(2):
                            nc.tensor.matmul(
                                ps, aoh[:, 2 * hp:2 * hp + 2, ts:ts + 128],
                                woh_sb[:, 2 * hp:2 * hp + 2, n0:n0 + 512],
                                start=(hp == 0), stop=False, perf_mode=DR)
                        for hp in range(2):
                            nc.tensor.matmul(
                                ps, aol[:, 2 * hp:2 * hp + 2, ts:ts + 128],
                                woh_sb[:, 2 * hp:2 * hp + 2, n0:n0 + 512],
                                start=False, stop=False, perf_mode=DR)
                        for hp in range(2):
                            nc.tensor.matmul(
                                ps, aoh[:, 2 * hp:2 * hp + 2, ts:ts + 128],
                                wol_sb[:, 2 * hp:2 * hp + 2, n0:n0 + 512],
                                start=False, stop=(hp == 1), perf_mode=DR)
                        osb = xsbpool.tile([128, 512], BF16, tag="osb",
                                           name="osb")
                        if split_copies and tt == c * 4 + 3:
                            nc.scalar.copy(osb[:, :256], ps[:, :256])
                            nc.vector.tensor_copy(osb[:, 256:], ps[:, 256:])
                        elif (tt * 8 + n0 // 512) % 2 == 0:
                            nc.scalar.copy(osb, ps)
                        else:
                            nc.vector.tensor_copy(osb, ps)
                        nc.sync.dma_start(out.ap()[ts:ts + 128, n0:n0 + 512],
                                          osb)
                        yield

            def drain(gen, n=10 ** 9):
                """Pull up to n units; True if the generator is exhausted."""
                for _ in range(n):
                    if next(gen, _SENTINEL) is _SENTINEL:
                        return True
                return False

            _SENTINEL = object()

            def hs_dma_closures(c):
                """Allocate next chunk's hs tiles; return deferred DMA
                emitters so the transfers can be paced into the O-proj
                stream (fair-sharing the DMA engines with osb writes)."""
                hsh_sb = hpool.tile([128, KT, CH], F8, tag="hsh", name="hsh_sb")
                hsl_sb = hpool.tile([128, KT, CH], F8, tag="hsl", name="hsl_sb")

                def mk(dst, src, g):
                    return lambda: nc.sync.dma_start(
                        dst[:, g * 8:(g + 1) * 8, :],
                        src[:, c, g * 8:(g + 1) * 8, :])

                fs = [mk(hsh_sb, hsh_d.ap(), g) for g in range(4)]
                fs += [mk(hsl_sb, hsl_d.ap(), g) for g in range(4)]
                return (hsh_sb, hsl_sb), fs

            # --- main schedule ---
            hs_cur = emit_hs_dmas(0, lo=False)
            for c in range(NCHUNK):
                hsh_sb, hsl_sb = hs_cur
                t0 = c * CH
                b, qb = c // 2, c % 2
                per_batch = 2 if qb else 1   # group units per Q quarter-batch

                def st(w):
                    return lambda kp: w[:, 2 * kp:2 * kp + 2, :]

                def mv(x):
                    return lambda kp: x[:, 2 * kp:2 * kp + 2, :]

                def mm(ps, s, v, start=False, stop=False):
                    nc.tensor.matmul(ps, s, v, start=start, stop=stop,
                                     perf_mode=DR)

                if c == 0:
                    # Term-staged startup: all hsl-free terms of K/V/Q0 run
                    # while hsl and later weights are still in flight.
                    kh, kl = st(wkh_t), st(wkl_t)
                    vh, vl = st(wvh_t), st(wvl_t)
                    qh, ql = st(wqh_t[0]), st(wql_t[0])
                    mh, ml = mv(hsh_sb), mv(hsl_sb)
                    psK = wpool.tile([128, CH], F32, tag="work", name="ps_k")
                    for kp in range(KP):
                        mm(psK, kh(kp), mh(kp), start=(kp == 0))
                    late_consts(hsl_sb)
                    for kp in range(KP):
                        mm(psK, kl(kp), mh(kp))
                    psV = []
                    for vi in range(CH // 128):
                        v0 = vi * 128
                        pv = wpool.tile([128, HD], F32, tag="work",
                                        name="ps_v")
                        for kp in range(KP):
                            mm(pv, hsh_sb[:, 2 * kp:2 * kp + 2, v0:v0 + 128],
                               vh(kp), start=(kp == 0))
                        for kp in range(KP):
                            mm(pv, hsh_sb[:, 2 * kp:2 * kp + 2, v0:v0 + 128],
                               vl(kp))
                        psV.append(pv)
                    psQ = wpool.tile([128, CH], F32, tag="work", name="ps_q")
                    for kp in range(KP):
                        mm(psQ, qh(kp), mh(kp), start=(kp == 0))
                    for kp in range(KP):
                        mm(psQ, ql(kp), mh(kp))
                    # stage B: hsl-dependent third terms
                    for kp in range(KP):
                        mm(psK, kh(kp), ml(kp), stop=(kp == KP - 1))
                    rope(psK, kt_rot[:, t0:t0 + CH],
                         ck_sb[:, t0:t0 + CH], sk_sb[:, t0:t0 + CH])
                    for vi in range(CH // 128):
                        v0 = vi * 128
                        for kp in range(KP):
                            mm(psV[vi],
                               hsl_sb[:, 2 * kp:2 * kp + 2, v0:v0 + 128],
                               vh(kp), stop=(kp == KP - 1))
                        nc.scalar.copy(v_sb[:, t0 // 128 + vi, :], psV[vi])
                    for kp in range(KP):
                        mm(psQ, qh(kp), ml(kp), stop=(kp == KP - 1))
                    rope(psQ, qt_rot[:, 0, t0:t0 + CH],
                         cq_sb[:, t0:t0 + CH], sq_sb[:, t0:t0 + CH])
                    m_start = 1
                    active = None
                    created = 0
                else:
                    # K projection first (its rope unblocks all groups)
                    ps = wpool.tile([128, CH], F32, tag="work", name="ps_k")
                    mm3(ps, st(wkh_t), st(wkl_t), mv(hsh_sb), mv(hsl_sb))
                    rope(ps, kt_rot[:, t0:t0 + CH],
                         ck_sb[:, t0:t0 + CH], sk_sb[:, t0:t0 + CH])
                    # V projection
                    for vi in range(CH // 128):
                        tt = t0 // 128 + vi
                        ps = wpool.tile([128, HD], F32, tag="work",
                                        name="ps_v")
                        v0 = vi * 128
                        mm3(ps,
                            lambda kp: hsh_sb[:, 2 * kp:2 * kp + 2,
                                              v0:v0 + 128],
                            lambda kp: hsl_sb[:, 2 * kp:2 * kp + 2,
                                              v0:v0 + 128],
                            st(wvh_t), st(wvl_t))
                        nc.scalar.copy(v_sb[:, tt, :], ps)
                    m_start = 0
                    active = None
                    created = 0
                # Q heads in quarter-batches; group h-1's units interleave
                # into head h's matmul stream (PE-heavy, Act-light)
                for m in range(m_start, HL):
                    ps = wpool.tile([128, CH], F32, tag="work", name="ps_q")
                    sh, sl = st(wqh_t[m]), st(wql_t[m])
                    mh, ml = mv(hsh_sb), mv(hsl_sb)
                    for bi in range(4):
                        k0, k1 = bi * 4, bi * 4 + 4
                        for kp in range(k0, k1):
                            nc.tensor.matmul(ps, sh(kp), mh(kp),
                                             start=(kp == 0), stop=False,
                                             perf_mode=DR)
                        for kp in range(k0, k1):
                            nc.tensor.matmul(ps, sl(kp), mh(kp),
                                             start=False, stop=False,
                                             perf_mode=DR)
                        for kp in range(k0, k1):
                            nc.tensor.matmul(ps, sh(kp), ml(kp), start=False,
                                             stop=(bi == 3 and kp == k1 - 1),
                                             perf_mode=DR)
                        if active is not None:
                            drain(active, per_batch)
                    rope(ps, qt_rot[:, m, t0:t0 + CH],
                         cq_sb[:, t0:t0 + CH], sq_sb[:, t0:t0 + CH])
                    if created < m:
                        if active is not None:
                            drain(active)
                        active = group_units(b, created, qb)
                        created += 1
                # remaining groups (h=2,3) interleave with the O projection
                # of the previous chunk; chunk 0 has no O-proj to interleave.
                # Next-chunk hs DMAs (and wo, at c==0) pace into the stream.
                tail_gens = [active, group_units(b, HL - 1, qb)]
                active = None
                feed = []
                if c + 1 < NCHUNK:
                    hs_cur, feed = hs_dma_closures(c + 1)
                if c == 0:
                    feed.append(lambda: nc.sync.dma_start(woh_sb, woh_d.ap()))
                    feed.append(lambda: nc.sync.dma_start(wol_sb, wol_d.ap()))
                if c >= 1:
                    op = oproj_units(c - 1)
                    gi = 0
                    done_op = False
                    opn = 0
                    while not done_op:
                        if gi < len(tail_gens):
                            if drain(tail_gens[gi], 1):
                                gi += 1
                                continue
                        done_op = drain(op, 2 if qb else 4)
                        opn += 1
                        if feed and opn % 2 == 0:
                            feed.pop(0)()
                    for g in tail_gens[gi:]:
                        drain(g)
                else:
                    for f in feed:
                        f()
                    feed = []
                    for g in tail_gens:
                        drain(g)
                for f in feed:
                    f()
                if c == NCHUNK - 1 and pending[0] is not None:
                    # flush the last epilogue now: its DVE/Act ops run while
                    # the PE works through the final O projection below
                    epilogue(pending[0])
                    pending[0] = None
            drain(oproj_units(NCHUNK - 1, split_copies=True))
    nc.compile()
    return nc


def _get_nc():
    if "nc" not in _CACHE:
        _CACHE["nc"] = _build()
    return _CACHE["nc"]


def _split8(x: np.ndarray):
    """Split f32 array into (hi, lo) e4m3 pair with hi + lo ~= x."""
    hi = x.astype(E4M3)
    lo = (x - hi.astype(np.float32)).astype(E4M3)
    return hi, lo


def _prep_inputs(inputs) -> list[dict]:
    bf16 = ml_dtypes.bfloat16
    hs = np.asarray(inputs["hidden_states"], dtype=np.float32).reshape(T, DIM)
    hsT = np.ascontiguousarray(hs.T)
    hsh, hsl = _split8(hsT)

    def swz_hs(x):  # [DIM, T] -> [128, NCHUNK, KT, CH] (SBUF layout)
        return np.ascontiguousarray(
            x.reshape(KT, 128, NCHUNK, CH).transpose(1, 2, 0, 3))

    hsh = swz_hs(hsh)
    hsl = swz_hs(hsl)

    fc = np.asarray(inputs["freqs_cos"], dtype=np.float32).reshape(T, HD // 2).T
    fs = np.asarray(inputs["freqs_sin"], dtype=np.float32).reshape(T, HD // 2).T
    cos2 = np.concatenate([fc, fc], axis=0)            # [128, T]
    sin2 = np.concatenate([-fs, fs], axis=0)           # signed half-rotation
    cos_qv = np.ascontiguousarray(cos2 * (SCALE / S_Q)).astype(bf16)
    sin_qv = np.ascontiguousarray(sin2 * (SCALE / S_Q)).astype(bf16)
    cos_kv = np.ascontiguousarray(cos2 * (1.0 / S_K)).astype(bf16)
    sin_kv = np.ascontiguousarray(sin2 * (1.0 / S_K)).astype(bf16)

    maskT = np.ascontiguousarray(
        np.asarray(inputs["attention_mask"], dtype=np.float32)[0, 0, :128, :128].T)

    perm = np.concatenate([np.arange(0, HD, 2), np.arange(1, HD, 2)])
    Wq = np.asarray(inputs["Wq"], dtype=np.float32)
    Wk = np.asarray(inputs["Wk"], dtype=np.float32)
    Wv = np.asarray(inputs["Wv"], dtype=np.float32)
    Wo = np.asarray(inputs["Wo"], dtype=np.float32)

    def swz_w(x, nh):  # [DIM, nh*HD] -> [128, nh, KT, HD]
        return np.ascontiguousarray(
            x.reshape(KT, 128, nh, HD).transpose(1, 2, 0, 3))

    in_maps = []
    for c in range(N_CORES):
        wq_c = np.concatenate(
            [Wq[:, (c * HL + h) * HD:(c * HL + h + 1) * HD][:, perm]
             for h in range(HL)], axis=1) * S_Q
        wk_c = Wk[:, c * HD:(c + 1) * HD][:, perm] * S_K
        wv_c = Wv[:, c * HD:(c + 1) * HD] * S_V
        wo_c = Wo[c * HL * HD:(c + 1) * HL * HD, :] * S_O
        wqh, wql = _split8(wq_c)
        wkh, wkl = _split8(wk_c)
        wvh, wvl = _split8(wv_c)
        woh, wol = _split8(wo_c)
        in_maps.append({
            "hsh": hsh, "hsl": hsl,
            "wqh": swz_w(wqh, HL), "wql": swz_w(wql, HL),
            "wkh": swz_w(wkh, 1).reshape(128, KT, HD),
            "wkl": swz_w(wkl, 1).reshape(128, KT, HD),
            "wvh": swz_w(wvh, 1).reshape(128, KT, HD),
            "wvl": swz_w(wvl, 1).reshape(128, KT, HD),
            "woh": np.ascontiguousarray(
                woh.reshape(HL, 128, DIM).transpose(1, 0, 2)),
            "wol": np.ascontiguousarray(
                wol.reshape(HL, 128, DIM).transpose(1, 0, 2)),
            "cos_q": cos_qv, "sin_q": sin_qv,
            "cos_k": cos_kv, "sin_k": sin_kv,
            "maskT": maskT,
        })
    return in_maps


def kernel(**inputs) -> np.ndarray:
    nc = _get_nc()
    in_maps = _prep_inputs(inputs)
    res = bass_utils.run_bass_kernel_spmd(nc, in_maps,
                                          core_ids=list(range(N_CORES)))
    acc = np.zeros((T, DIM), dtype=np.float32)
    for c in range(N_CORES):
        acc += np.asarray(res.results[c]["out"], dtype=np.float32)
    return (acc * (1.0 / (S_V * S_O))).reshape(B, S, DIM)


# revision 61
# speedup vs baseline: 1.3017x; 1.0016x over previous
"""Trainium2 Bass kernel for MllamaTextSdpaAttention (GQA + RoPE + causal SDPA).

Strategy: tensor-parallel over heads across 8 NeuronCores. Core c owns
q-heads [4c, 4c+4) and kv-head c (kv groups intact). Each core computes
hidden @ Wq/Wk/Wv slices, RoPE, causal attention for its heads, and its
row-slice of the Wo matmul, yielding a partial [T, DIM] output (bf16).
The host sums the 8 partials in f32.

Key techniques:
- All four projections (Q/K/V/O) run on the PE in fp8e4m3 DoubleRow mode
  (2 k-tiles of contraction per instruction at 0.5 cycles/column = 4x the
  bf16 FLOP rate). Accuracy is preserved with a 3-term residual split:
  each operand X is split (host-side for inputs/weights, on-device for
  ao) into Xh = fp8(X), Xl = fp8(X - Xh), and W@X ~= Wh@Xh + Wl@Xh +
  Wh@Xl. Net cost: 0.75x the bf16 column count. Weights are pre-scaled
  (x32 Wq/Wk/Wo, x16 Wv) into e4m3's normal range; descales fold into
  the RoPE tables and the host-side gather.
- All inputs are pre-swizzled on the host into exact SBUF layouts so
  every DMA moves >=512-byte contiguous runs (full 360 GB/s; under 512B
  the DMA engines run at half rate).
- Attention stays bf16: transposed scores (scT = K_rot^T.T @ Q_rot^T),
  exp on Act feeds P@V directly, rowsums via ones-vector matmul, 1/rowsum
  applied to the small out^T tile via GpSimd partition-broadcast. RoPE
  as a half-rotation with host-permuted weight columns. Causality at
  128-block granularity.
- Schedule: per chunk, K and V projections run first, then Q heads with
  attention groups interleaved one head behind (group h emits after
  Q_{h+1}), so DVE RoPE latency and chunk-0's DMA-bound startup overlap
  with PE work. The O projection for chunk c-1's tokens is emitted at the
  end of chunk c (ao complete once chunk c's first group fires the
  deferred epilogue), spreading output DMA across the kernel and leaving
  only chunk 3's O-proj after the last attention group.
"""

import numpy as np
import ml_dtypes

import concourse.bacc as bacc
import concourse.bass as bass
import concourse.bass_isa as bass_isa
import concourse.mybir as mybir
from concourse.tile import TileContext
from concourse import bass_utils

BF16 = mybir.dt.bfloat16
F32 = mybir.dt.float32
F8 = mybir.dt.float8e4
E4M3 = ml_dtypes.float8_e4m3

B, S, DIM = 2, 1024, 4096
T = B * S                     # 2048 tokens, batch-major
N_HEADS, N_KV = 32, 8
HD = 128                      # head dim == partition count
N_CORES = 8
HL = N_HEADS // N_CORES       # 4 local q-heads per core
KT = DIM // 128               # 32 feature tiles
KP = KT // 2                  # 16 k-tile PAIRS (DoubleRow)
CH = 512                      # projection token-chunk
NCHUNK = T // CH
QB = 512                      # attention q-block width
TT = T // 128                 # 16 token tiles global
SCALE = 1.0 / float(np.sqrt(HD))
S_Q = 32.0                    # weight pre-scales for fp8 range
S_K = 32.0
S_V = 16.0
S_O = 32.0
DR = mybir.MatmulPerfMode.DoubleRow

_CACHE: dict = {}


def _build():
    nc = bacc.Bacc("TRN2", target_bir_lowering=False, debug=False,
                   enable_asserts=False, dynamic_dma_scratch_size=2048)

    # all tensors pre-swizzled host-side into SBUF layout (partition-major)
    hsh_d = nc.dram_tensor("hsh", [128, NCHUNK, KT, CH], F8, kind="ExternalInput")
    hsl_d = nc.dram_tensor("hsl", [128, NCHUNK, KT, CH], F8, kind="ExternalInput")
    wqh_d = nc.dram_tensor("wqh", [128, HL, KT, HD], F8, kind="ExternalInput")
    wql_d = nc.dram_tensor("wql", [128, HL, KT, HD], F8, kind="ExternalInput")
    wkh_d = nc.dram_tensor("wkh", [128, KT, HD], F8, kind="ExternalInput")
    wkl_d = nc.dram_tensor("wkl", [128, KT, HD], F8, kind="ExternalInput")
    wvh_d = nc.dram_tensor("wvh", [128, KT, HD], F8, kind="ExternalInput")
    wvl_d = nc.dram_tensor("wvl", [128, KT, HD], F8, kind="ExternalInput")
    woh_d = nc.dram_tensor("woh", [128, HL, DIM], F8, kind="ExternalInput")
    wol_d = nc.dram_tensor("wol", [128, HL, DIM], F8, kind="ExternalInput")
    cos_q = nc.dram_tensor("cos_q", [HD, T], BF16, kind="ExternalInput")
    sin_q = nc.dram_tensor("sin_q", [HD, T], BF16, kind="ExternalInput")
    cos_k = nc.dram_tensor("cos_k", [HD, T], BF16, kind="ExternalInput")
    sin_k = nc.dram_tensor("sin_k", [HD, T], BF16, kind="ExternalInput")
    maskT = nc.dram_tensor("maskT", [128, 128], F32, kind="ExternalInput")
    out = nc.dram_tensor("out", [T, DIM], BF16, kind="ExternalOutput")

    Exp = mybir.ActivationFunctionType.Exp

    with TileContext(nc) as tc:
        with tc.tile_pool(name="consts", bufs=1) as cpool, \
             tc.tile_pool(name="hs", bufs=2) as hpool, \
             tc.tile_pool(name="rope_tmp", bufs=1) as rpool, \
             tc.tile_pool(name="work_ps", bufs=6, space=bass.MemorySpace.PSUM) as wpool, \
             tc.tile_pool(name="ot_ps", bufs=2, space=bass.MemorySpace.PSUM) as otpool, \
             tc.tile_pool(name="et", bufs=5) as epool, \
             tc.tile_pool(name="esum", bufs=2) as espool, \
             tc.tile_pool(name="tao", bufs=1) as taopool, \
             tc.tile_pool(name="out_sb", bufs=6) as xsbpool:

            wqh_t = [cpool.tile([128, KT, HD], F8, tag=f"wqh{m}", name=f"wqh{m}")
                     for m in range(HL)]
            wql_t = [cpool.tile([128, KT, HD], F8, tag=f"wql{m}", name=f"wql{m}")
                     for m in range(HL)]
            wkh_t = cpool.tile([128, KT, HD], F8, tag="wkh")
            wkl_t = cpool.tile([128, KT, HD], F8, tag="wkl")
            wvh_t = cpool.tile([128, KT, HD], F8, tag="wvh")
            wvl_t = cpool.tile([128, KT, HD], F8, tag="wvl")
            woh_sb = cpool.tile([128, HL, DIM], F8, tag="woh")
            wol_sb = cpool.tile([128, HL, DIM], F8, tag="wol")
            cq_sb = cpool.tile([128, T], BF16, tag="cq")
            sq_sb = cpool.tile([128, T], BF16, tag="sq")
            ck_sb = cpool.tile([128, T], BF16, tag="ck")
            sk_sb = cpool.tile([128, T], BF16, tag="sk")
            maskT_sb = cpool.tile([128, 128], F32, tag="maskT")
            qt_rot = cpool.tile([128, HL, T], BF16, tag="qt")
            kt_rot = cpool.tile([128, T], BF16, tag="kt")
            v_sb = cpool.tile([128, TT, HD], BF16, tag="v")
            aoh = cpool.tile([128, HL, T], F8, tag="aoh")
            aol = cpool.tile([128, HL, T], F8, tag="aol")

            # startup-critical DMA first: K-projection weights
            nc.sync.dma_start(wkh_t[:, 0:8, :], wkh_d.ap()[:, 0:8, :])
            nc.sync.dma_start(wkh_t[:, 8:KT, :], wkh_d.ap()[:, 8:KT, :])
            nc.sync.dma_start(wkl_t, wkl_d.ap())

            def emit_hs_dmas(c, lo=True):
                hsh_sb = hpool.tile([128, KT, CH], F8, tag="hsh", name="hsh_sb")
                hsl_sb = hpool.tile([128, KT, CH], F8, tag="hsl", name="hsl_sb")
                for g in range(4):
                    nc.sync.dma_start(hsh_sb[:, g * 8:(g + 1) * 8, :],
                                      hsh_d.ap()[:, c, g * 8:(g + 1) * 8, :])
                if lo:
                    for g in range(4):
                        nc.sync.dma_start(hsl_sb[:, g * 8:(g + 1) * 8, :],
                                          hsl_d.ap()[:, c, g * 8:(g + 1) * 8, :])
                return hsh_sb, hsl_sb

            def late_consts(hsl_sb):
                # strictly ordered by first use under the term-staged chunk-0
                # emission: V terms, then K-hl/V-lh (hsl), then Q0, ropes, Q1+
                nc.sync.dma_start(wvh_t, wvh_d.ap())
                nc.sync.dma_start(wvl_t, wvl_d.ap())
                nc.sync.dma_start(wqh_t[0], wqh_d.ap()[:, 0])
                nc.sync.dma_start(wql_t[0], wql_d.ap()[:, 0])
                nc.sync.dma_start(cq_sb, cos_q.ap())
                nc.sync.dma_start(sq_sb, sin_q.ap())
                nc.sync.dma_start(maskT_sb, maskT.ap())
                for g in range(4):
                    nc.sync.dma_start(hsl_sb[:, g * 8:(g + 1) * 8, :],
                                      hsl_d.ap()[:, 0, g * 8:(g + 1) * 8, :])
                nc.sync.dma_start(ck_sb, cos_k.ap())
                nc.sync.dma_start(sk_sb, sin_k.ap())
                nc.sync.dma_start(wqh_t[1], wqh_d.ap()[:, 1])
                nc.sync.dma_start(wql_t[1], wql_d.ap()[:, 1])
                for m in range(2, HL):
                    nc.sync.dma_start(wqh_t[m], wqh_d.ap()[:, m])
                    nc.sync.dma_start(wql_t[m], wql_d.ap()[:, m])

            def rope(ps, out_ap, cos_ap, sin_ap):
                """out = ps*cos + halfswap(ps)*sin  (signs baked into sin)."""
                t1 = rpool.tile([128, CH], F32, tag="r1", name="t1")
                t2 = rpool.tile([128, CH], F32, tag="r2", name="t2")
                nc.vector.tensor_mul(t1, ps, cos_ap)
                nc.vector.tensor_mul(t2[0:64, :], ps[64:128, :], sin_ap[0:64, :])
                nc.vector.tensor_mul(t2[64:128, :], ps[0:64, :], sin_ap[64:128, :])
                nc.vector.tensor_add(out_ap, t1, t2)

            def mm3(ps, st_h, st_l, mv_h, mv_l):
                """3-term fp8 DoubleRow accumulation over all KT k-tiles."""
                for kp in range(KP):
                    nc.tensor.matmul(ps, st_h(kp), mv_h(kp),
                                     start=(kp == 0), stop=False, perf_mode=DR)
                for kp in range(KP):
                    nc.tensor.matmul(ps, st_l(kp), mv_h(kp),
                                     start=False, stop=False, perf_mode=DR)
                for kp in range(KP):
                    nc.tensor.matmul(ps, st_h(kp), mv_l(kp),
                                     start=False, stop=(kp == KP - 1),
                                     perf_mode=DR)

            # --- attention group machinery (transposed-scores scheme) ---
            pending = [None]

            def epilogue(st):
                rs, ot, h, q0 = st
                with nc.allow_low_precision("softmax rowsum recip in bf16"):
                    nc.vector.reciprocal(rs, rs)
                t = taopool.tile([128, QB], F32, tag="tao", name="tao")
                nc.vector.tensor_mul(t, ot, rs)
                nc.scalar.copy(aoh[:, h, q0:q0 + QB], t)
                nc.vector.tensor_sub(aol[:, h, q0:q0 + QB], t,
                                     aoh[:, h, q0:q0 + QB])

            def group_units(b, h, qb):
                """Generator: one yield per consumed score k-tile, so group
                work (Act-heavy exp) can be interleaved into PE-heavy Q/O
                projection streams."""
                q0 = b * S + qb * QB
                n_kt = (qb + 1) * (QB // 128)
                # esum accumulates sum_kt et_kt elementwise on DVE (bf16, 2x
                # mode); the final GpSimd partition_all_reduce turns it into
                # softmax rowsums broadcast across partitions. Keeps the
                # rowsum off the PE; bf16 accumulation costs ~0.5% on rs,
                # well inside the error budget.
                esum = espool.tile([128, QB], BF16, tag="esum", name="esum")
                ot = otpool.tile([128, QB], F32, tag="ot", name="ot")
                ets = [None] * n_kt

                def emit_sc(kt):
                    c0 = max(0, kt - qb * (QB // 128)) * 128
                    sc = wpool.tile([128, QB], F32, tag="work", name="sc")
                    nc.tensor.matmul(
                        sc[:, c0:],
                        kt_rot[:, b * S + kt * 128:b * S + (kt + 1) * 128],
                        qt_rot[:, h, q0 + c0:q0 + QB],
                        start=True, stop=True)
                    jd = kt - qb * (QB // 128)
                    if 0 <= jd < QB // 128:
                        nc.vector.tensor_add(sc[:, jd * 128:(jd + 1) * 128],
                                             sc[:, jd * 128:(jd + 1) * 128],
                                             maskT_sb)
                    et = epool.tile([128, QB], BF16, tag="et", name="et")
                    nc.scalar.activation(et[:, c0:], sc[:, c0:], Exp,
                                         bias=0.0, scale=1.0)
                    ets[kt] = (et, c0)

                def consume(kt):
                    et, c0 = ets[kt]
                    if kt == 0:
                        nc.vector.tensor_copy(esum, et)
                    else:
                        nc.vector.tensor_add(esum[:, c0:], esum[:, c0:],
                                             et[:, c0:])
                    nc.tensor.matmul(ot[:, c0:], v_sb[:, b * (S // 128) + kt, :],
                                     et[:, c0:], start=(kt == 0),
                                     stop=(kt == n_kt - 1))
                    ets[kt] = None
                    if kt == 0 and pending[0] is not None:
                        epilogue(pending[0])
                        pending[0] = None

                for kt in range(n_kt):
                    emit_sc(kt)
                    if kt >= 2:
                        consume(kt - 2)
                        yield
                for kt in range(max(0, n_kt - 2), n_kt):
                    consume(kt)
                    yield
                nc.gpsimd.partition_all_reduce(esum, esum, 128,
                                               bass_isa.ReduceOp.add)
                pending[0] = (esum, ot, h, q0)

            def oproj_units(c, direct_out=False):
                """Generator: one yield per O-projection psum tile (fp8
                DoubleRow) for chunk c's 4 token tiles."""
                for tt in range(c * 4, c * 4 + 4):
                    ts = tt * 128
                    for n0 in range(0, DIM, 512):
                        ps = wpool.tile([128, 512], F32, tag="work", name="ps_o")
                        # hp-outer order: the head-pair (0,1) terms of each
                        # tile run before any (2,3) term, covering the last
                        # group's epilogue-chain latency with real work
                        for hp in range(2):
                            h2 = 2 * hp
                            nc.tensor.matmul(
                                ps, aoh[:, h2:h2 + 2, ts:ts + 128],
                                woh_sb[:, h2:h2 + 2, n0:n0 + 512],
                                start=(hp == 0), stop=False, perf_mode=DR)
                            nc.tensor.matmul(
                                ps, aol[:, h2:h2 + 2, ts:ts + 128],
                                woh_sb[:, h2:h2 + 2, n0:n0 + 512],
                                start=False, stop=False, perf_mode=DR)
                            nc.tensor.matmul(
                                ps, aoh[:, h2:h2 + 2, ts:ts + 128],
                                wol_sb[:, h2:h2 + 2, n0:n0 + 512],
                                start=False, stop=(hp == 1), perf_mode=DR)
                        osb = xsbpool.tile([128, 512], BF16, tag="osb",
                                           name="osb")
                        # alternate copy engines AND DMA-issue paths (HWDGE
                        # via SP vs SWDGE via Pool) so neither serializes the
                        # psum->sbuf->dram drain
                        if (tt * 8 + n0 // 512) % 2 == 0:
                            nc.scalar.copy(osb, ps)
                            nc.sync.dma_start(
                                out.ap()[ts:ts + 128, n0:n0 + 512], osb)
                        else:
                            nc.vector.tensor_copy(osb, ps)
                            nc.gpsimd.dma_start(
                                out.ap()[ts:ts + 128, n0:n0 + 512], osb)
                        yield

            def drain(gen, n=10 ** 9):
                """Pull up to n units; True if the generator is exhausted."""
                for _ in range(n):
                    if next(gen, _SENTINEL) is _SENTINEL:
                        return True
                return False

            _SENTINEL = object()

            def hs_dma_closures(c):
                """Allocate next chunk's hs tiles; return deferred DMA
                emitters so the transfers can be paced into the O-proj
                stream (fair-sharing the DMA engines with osb writes)."""
                hsh_sb = hpool.tile([128, KT, CH], F8, tag="hsh", name="hsh_sb")
                hsl_sb = hpool.tile([128, KT, CH], F8, tag="hsl", name="hsl_sb")

                def mk(dst, src, g):
                    return lambda: nc.sync.dma_start(
                        dst[:, g * 8:(g + 1) * 8, :],
                        src[:, c, g * 8:(g + 1) * 8, :])

                fs = [mk(hsh_sb, hsh_d.ap(), g) for g in range(4)]
                fs += [mk(hsl_sb, hsl_d.ap(), g) for g in range(4)]
                return (hsh_sb, hsl_sb), fs

            # --- main schedule ---
            hs_cur = emit_hs_dmas(0, lo=False)
            for c in range(NCHUNK):
                hsh_sb, hsl_sb = hs_cur
                t0 = c * CH
                b, qb = c // 2, c % 2
                per_batch = 2 if qb else 1   # group units per Q quarter-batch

                def st(w):
                    return lambda kp: w[:, 2 * kp:2 * kp + 2, :]

                def mv(x):
                    return lambda kp: x[:, 2 * kp:2 * kp + 2, :]

                def mm(ps, s, v, start=False, stop=False):
                    nc.tensor.matmul(ps, s, v, start=start, stop=stop,
                                     perf_mode=DR)

                if c == 0:
                    # Term-staged startup, emission ordered to match DMA
                    # arrival: K-hh/lh (wk+hsh), V-hh/hl (wv), K-hl & V-lh
                    # (hsl), then Q0 (wq0), so the PE is never waiting on a
                    # transfer that sits behind unneeded bytes.
                    kh, kl = st(wkh_t), st(wkl_t)
                    vh, vl = st(wvh_t), st(wvl_t)
                    qh, ql = st(wqh_t[0]), st(wql_t[0])
                    mh, ml = mv(hsh_sb), mv(hsl_sb)
                    psK = wpool.tile([128, CH], F32, tag="work", name="ps_k")
                    for kp in range(KP):
                        mm(psK, kh(kp), mh(kp), start=(kp == 0))
                    late_consts(hsl_sb)
                    for kp in range(KP):
                        mm(psK, kl(kp), mh(kp))
                    psV = []
                    for vi in range(CH // 128):
                        v0 = vi * 128
                        pv = wpool.tile([128, HD], F32, tag="work",
                                        name="ps_v")
                        for kp in range(KP):
                            mm(pv, hsh_sb[:, 2 * kp:2 * kp + 2, v0:v0 + 128],
                               vh(kp), start=(kp == 0))
                        for kp in range(KP):
                            mm(pv, hsh_sb[:, 2 * kp:2 * kp + 2, v0:v0 + 128],
                               vl(kp))
                        psV.append(pv)
                    psQ = wpool.tile([128, CH], F32, tag="work", name="ps_q")
                    for kp in range(KP):
                        mm(psQ, qh(kp), mh(kp), start=(kp == 0))
                    for kp in range(KP):
                        mm(psQ, ql(kp), mh(kp))
                    # stage B: hsl-dependent third terms
                    for kp in range(KP):
                        mm(psK, kh(kp), ml(kp), stop=(kp == KP - 1))
                    rope(psK, kt_rot[:, t0:t0 + CH],
                         ck_sb[:, t0:t0 + CH], sk_sb[:, t0:t0 + CH])
                    for vi in range(CH // 128):
                        v0 = vi * 128
                        for kp in range(KP):
                            mm(psV[vi],
                               hsl_sb[:, 2 * kp:2 * kp + 2, v0:v0 + 128],
                               vh(kp), stop=(kp == KP - 1))
                        nc.scalar.copy(v_sb[:, t0 // 128 + vi, :], psV[vi])
                    for kp in range(KP):
                        mm(psQ, qh(kp), ml(kp), stop=(kp == KP - 1))
                    rope(psQ, qt_rot[:, 0, t0:t0 + CH],
                         cq_sb[:, t0:t0 + CH], sq_sb[:, t0:t0 + CH])
                    m_start = 1
                    active = None
                    created = 0
                else:
                    # K projection first (its rope unblocks all groups)
                    ps = wpool.tile([128, CH], F32, tag="work", name="ps_k")
                    mm3(ps, st(wkh_t), st(wkl_t), mv(hsh_sb), mv(hsl_sb))
                    rope(ps, kt_rot[:, t0:t0 + CH],
                         ck_sb[:, t0:t0 + CH], sk_sb[:, t0:t0 + CH])
                    # V projection
                    for vi in range(CH // 128):
                        tt = t0 // 128 + vi
                        ps = wpool.tile([128, HD], F32, tag="work",
                                        name="ps_v")
                        v0 = vi * 128
                        mm3(ps,
                            lambda kp: hsh_sb[:, 2 * kp:2 * kp + 2,
                                              v0:v0 + 128],
                            lambda kp: hsl_sb[:, 2 * kp:2 * kp + 2,
                                              v0:v0 + 128],
                            st(wvh_t), st(wvl_t))
                        nc.scalar.copy(v_sb[:, tt, :], ps)
                    m_start = 0
                    active = None
                    created = 0
                # Q heads in quarter-batches; group h-1's units interleave
                # into head h's matmul stream (PE-heavy, Act-light)
                for m in range(m_start, HL):
                    ps = wpool.tile([128, CH], F32, tag="work", name="ps_q")
                    sh, sl = st(wqh_t[m]), st(wql_t[m])
                    mh, ml = mv(hsh_sb), mv(hsl_sb)
                    nb = 8 if qb else 4
                    for bi in range(nb):
                        k0 = bi * KP // nb
                        k1 = (bi + 1) * KP // nb
                        for kp in range(k0, k1):
                            nc.tensor.matmul(ps, sh(kp), mh(kp),
                                             start=(kp == 0), stop=False,
                                             perf_mode=DR)
                        for kp in range(k0, k1):
                            nc.tensor.matmul(ps, sl(kp), mh(kp),
                                             start=False, stop=False,
                                             perf_mode=DR)
                        for kp in range(k0, k1):
                            nc.tensor.matmul(ps, sh(kp), ml(kp), start=False,
                                             stop=(bi == nb - 1
                                                   and kp == k1 - 1),
                                             perf_mode=DR)
                        if active is not None:
                            drain(active, 1)
                    rope(ps, qt_rot[:, m, t0:t0 + CH],
                         cq_sb[:, t0:t0 + CH], sq_sb[:, t0:t0 + CH])
                    if created < m:
                        if active is not None:
                            drain(active)
                        active = group_units(b, created, qb)
                        created += 1
                # remaining groups (h=2,3) interleave with the O projection
                # of the previous chunk; chunk 0 has no O-proj to interleave.
                # Next-chunk hs DMAs (and wo, at c==0) pace into the stream.
                tail_gens = [active, group_units(b, HL - 1, qb)]
                active = None
                feed = []
                if c + 1 < NCHUNK:
                    hs_cur, feed = hs_dma_closures(c + 1)
                if c == 0:
                    feed.append(lambda: nc.sync.dma_start(woh_sb, woh_d.ap()))
                    feed.append(lambda: nc.sync.dma_start(wol_sb, wol_d.ap()))
                if c >= 1:
                    op = oproj_units(c - 1)
                    gi = 0
                    done_op = False
                    opn = 0
                    while not done_op:
                        if gi < len(tail_gens):
                            if drain(tail_gens[gi], 1):
                                gi += 1
                                continue
                        done_op = drain(op, 2 if qb else 4)
                        opn += 1
                        if feed and opn % 2 == 0:
                            feed.pop(0)()
                    for g in tail_gens[gi:]:
                        drain(g)
                else:
                    for f in feed:
                        f()
                    feed = []
                    for g in tail_gens:
                        drain(g)
                for f in feed:
                    f()
                if c == NCHUNK - 1 and pending[0] is not None:
                    # flush the last epilogue now: its DVE/Act ops run while
                    # the PE works through the final O projection below
                    epilogue(pending[0])
                    pending[0] = None
            drain(oproj_units(NCHUNK - 1))
    nc.compile()
    return nc


def _get_nc():
    if "nc" not in _CACHE:
        _CACHE["nc"] = _build()
    return _CACHE["nc"]


def _split8(x: np.ndarray):
    """Split f32 array into (hi, lo) e4m3 pair with hi + lo ~= x."""
    hi = x.astype(E4M3)
    lo = (x - hi.astype(np.float32)).astype(E4M3)
    return hi, lo


def _prep_inputs(inputs) -> list[dict]:
    bf16 = ml_dtypes.bfloat16
    hs = np.asarray(inputs["hidden_states"], dtype=np.float32).reshape(T, DIM)
    hsT = np.ascontiguousarray(hs.T)
    hsh, hsl = _split8(hsT)

    def swz_hs(x):  # [DIM, T] -> [128, NCHUNK, KT, CH] (SBUF layout)
        return np.ascontiguousarray(
            x.reshape(KT, 128, NCHUNK, CH).transpose(1, 2, 0, 3))

    hsh = swz_hs(hsh)
    hsl = swz_hs(hsl)

    fc = np.asarray(inputs["freqs_cos"], dtype=np.float32).reshape(T, HD // 2).T
    fs = np.asarray(inputs["freqs_sin"], dtype=np.float32).reshape(T, HD // 2).T
    cos2 = np.concatenate([fc, fc], axis=0)            # [128, T]
    sin2 = np.concatenate([-fs, fs], axis=0)           # signed half-rotation
    cos_qv = np.ascontiguousarray(cos2 * (SCALE / S_Q)).astype(bf16)
    sin_qv = np.ascontiguousarray(sin2 * (SCALE / S_Q)).astype(bf16)
    cos_kv = np.ascontiguousarray(cos2 * (1.0 / S_K)).astype(bf16)
    sin_kv = np.ascontiguousarray(sin2 * (1.0 / S_K)).astype(bf16)

    maskT = np.ascontiguousarray(
        np.asarray(inputs["attention_mask"], dtype=np.float32)[0, 0, :128, :128].T)

    perm = np.concatenate([np.arange(0, HD, 2), np.arange(1, HD, 2)])
    Wq = np.asarray(inputs["Wq"], dtype=np.float32)
    Wk = np.asarray(inputs["Wk"], dtype=np.float32)
    Wv = np.asarray(inputs["Wv"], dtype=np.float32)
    Wo = np.asarray(inputs["Wo"], dtype=np.float32)

    def swz_w(x, nh):  # [DIM, nh*HD] -> [128, nh, KT, HD]
        return np.ascontiguousarray(
            x.reshape(KT, 128, nh, HD).transpose(1, 2, 0, 3))

    in_maps = []
    for c in range(N_CORES):
        wq_c = np.concatenate(
            [Wq[:, (c * HL + h) * HD:(c * HL + h + 1) * HD][:, perm]
             for h in range(HL)], axis=1) * S_Q
        wk_c = Wk[:, c * HD:(c + 1) * HD][:, perm] * S_K
        wv_c = Wv[:, c * HD:(c + 1) * HD] * S_V
        wo_c = Wo[c * HL * HD:(c + 1) * HL * HD, :] * S_O
        wqh, wql = _split8(wq_c)
        wkh, wkl = _split8(wk_c)
        wvh, wvl = _split8(wv_c)
        woh, wol = _split8(wo_c)
        in_maps.append({
            "hsh": hsh, "hsl": hsl,
            "wqh": swz_w(wqh, HL), "wql": swz_w(wql, HL),
            "wkh": swz_w(wkh, 1).reshape(128, KT, HD),
            "wkl": swz_w(wkl, 1).reshape(128, KT, HD),
            "wvh": swz_w(wvh, 1).reshape(128, KT, HD),
            "wvl": swz_w(wvl, 1).reshape(128, KT, HD),
            "woh": np.ascontiguousarray(
                woh.reshape(HL, 128, DIM).transpose(1, 0, 2)),
            "wol": np.ascontiguousarray(
                wol.reshape(HL, 128, DIM).transpose(1, 0, 2)),
            "cos_q": cos_qv, "sin_q": sin_qv,
            "cos_k": cos_kv, "sin_k": sin_kv,
            "maskT": maskT,
        })
    return in_maps


def kernel(**inputs) -> np.ndarray:
    nc = _get_nc()
    in_maps = _prep_inputs(inputs)
    res = bass_utils.run_bass_kernel_spmd(nc, in_maps,
                                          core_ids=list(range(N_CORES)))
    acc = np.zeros((T, DIM), dtype=np.float32)
    for c in range(N_CORES):
        acc += np.asarray(res.results[c]["out"], dtype=np.float32)
    return (acc * (1.0 / (S_V * S_O))).reshape(B, S, DIM)


# revision 67
# speedup vs baseline: 1.3046x; 1.0022x over previous
"""Trainium2 Bass kernel for MllamaTextSdpaAttention (GQA + RoPE + causal SDPA).

Strategy: tensor-parallel over heads across 8 NeuronCores. Core c owns
q-heads [4c, 4c+4) and kv-head c (kv groups intact). Each core computes
hidden @ Wq/Wk/Wv slices, RoPE, causal attention for its heads, and its
row-slice of the Wo matmul, yielding a partial [T, DIM] output (bf16).
The host sums the 8 partials in f32.

Key techniques:
- All four projections (Q/K/V/O) run on the PE in fp8e4m3 DoubleRow mode
  (2 k-tiles of contraction per instruction at 0.5 cycles/column = 4x the
  bf16 FLOP rate). Accuracy is preserved with a 3-term residual split:
  each operand X is split (host-side for inputs/weights, on-device for
  ao) into Xh = fp8(X), Xl = fp8(X - Xh), and W@X ~= Wh@Xh + Wl@Xh +
  Wh@Xl. Net cost: 0.75x the bf16 column count. Weights are pre-scaled
  (x32 Wq/Wk/Wo, x16 Wv) into e4m3's normal range; descales fold into
  the RoPE tables and the host-side gather.
- All inputs are pre-swizzled on the host into exact SBUF layouts so
  every DMA moves >=512-byte contiguous runs (full 360 GB/s; under 512B
  the DMA engines run at half rate).
- Attention stays bf16 on the PE (scores + P@V only): transposed scores
  (scT = K_rot^T.T @ Q_rot^T), exp on Act feeds P@V directly. The
  softmax rowsums are computed OFF the PE: et tiles are summed
  elementwise on DVE (bf16, 2x/4x modes), then one GpSimd
  partition_all_reduce broadcasts the rowsum to all partitions; the
  reciprocal+normalize epilogue is deferred one group so the PE never
  waits on it. RoPE as a half-rotation with host-permuted weight
  columns. Causality at 128-block granularity.
- Schedule: per chunk, K and V run first (term-staged against DMA
  arrival for chunk 0), then Q heads emitted in sub-batches with the
  previous head's attention-group units interleaved into the stream
  (group work is Act-heavy, projections are PE-heavy). The O projection
  for chunk c-1 interleaves with chunk c's last two groups and paces
  the next chunk's hs DMAs, spreading output DMA across the kernel.
  Only chunk 3's O-proj trails the last attention group, with its
  epilogue chain hidden under chunk 2's O-proj.
"""

import numpy as np
import ml_dtypes

import concourse.bacc as bacc
import concourse.bass as bass
import concourse.bass_isa as bass_isa
import concourse.mybir as mybir
from concourse.tile import TileContext
from concourse import bass_utils

BF16 = mybir.dt.bfloat16
F32 = mybir.dt.float32
F8 = mybir.dt.float8e4
E4M3 = ml_dtypes.float8_e4m3

B, S, DIM = 2, 1024, 4096
T = B * S                     # 2048 tokens, batch-major
N_HEADS, N_KV = 32, 8
HD = 128                      # head dim == partition count
N_CORES = 8
HL = N_HEADS // N_CORES       # 4 local q-heads per core
KT = DIM // 128               # 32 feature tiles
KP = KT // 2                  # 16 k-tile PAIRS (DoubleRow)
CH = 512                      # projection token-chunk
NCHUNK = T // CH
QB = 512                      # attention q-block width
TT = T // 128                 # 16 token tiles global
SCALE = 1.0 / float(np.sqrt(HD))
S_Q = 32.0                    # weight pre-scales for fp8 range
S_K = 32.0
S_V = 16.0
S_O = 32.0
DR = mybir.MatmulPerfMode.DoubleRow

_CACHE: dict = {}


def _build():
    nc = bacc.Bacc("TRN2", target_bir_lowering=False, debug=False,
                   enable_asserts=False, dynamic_dma_scratch_size=2048)

    # all tensors pre-swizzled host-side into SBUF layout (partition-major)
    hsh_d = nc.dram_tensor("hsh", [128, NCHUNK, KT, CH], F8, kind="ExternalInput")
    hsl_d = nc.dram_tensor("hsl", [128, NCHUNK, KT, CH], F8, kind="ExternalInput")
    wqh_d = nc.dram_tensor("wqh", [128, HL, KT, HD], F8, kind="ExternalInput")
    wql_d = nc.dram_tensor("wql", [128, HL, KT, HD], F8, kind="ExternalInput")
    wkh_d = nc.dram_tensor("wkh", [128, KT, HD], F8, kind="ExternalInput")
    wkl_d = nc.dram_tensor("wkl", [128, KT, HD], F8, kind="ExternalInput")
    wvh_d = nc.dram_tensor("wvh", [128, KT, HD], F8, kind="ExternalInput")
    wvl_d = nc.dram_tensor("wvl", [128, KT, HD], F8, kind="ExternalInput")
    woh_d = nc.dram_tensor("woh", [128, HL, DIM], F8, kind="ExternalInput")
    wol_d = nc.dram_tensor("wol", [128, HL, DIM], F8, kind="ExternalInput")
    cos_q = nc.dram_tensor("cos_q", [HD, T], BF16, kind="ExternalInput")
    sin_q = nc.dram_tensor("sin_q", [HD, T], BF16, kind="ExternalInput")
    cos_k = nc.dram_tensor("cos_k", [HD, T], BF16, kind="ExternalInput")
    sin_k = nc.dram_tensor("sin_k", [HD, T], BF16, kind="ExternalInput")
    maskT = nc.dram_tensor("maskT", [128, 128], F32, kind="ExternalInput")
    out = nc.dram_tensor("out", [T, DIM], BF16, kind="ExternalOutput")

    Exp = mybir.ActivationFunctionType.Exp

    with TileContext(nc) as tc:
        with tc.tile_pool(name="consts", bufs=1) as cpool, \
             tc.tile_pool(name="hs", bufs=2) as hpool, \
             tc.tile_pool(name="rope_tmp", bufs=1) as rpool, \
             tc.tile_pool(name="work_ps", bufs=6, space=bass.MemorySpace.PSUM) as wpool, \
             tc.tile_pool(name="ot_ps", bufs=2, space=bass.MemorySpace.PSUM) as otpool, \
             tc.tile_pool(name="et", bufs=4) as epool, \
             tc.tile_pool(name="esum", bufs=1) as espool, \
             tc.tile_pool(name="tao", bufs=1) as taopool, \
             tc.tile_pool(name="out_sb", bufs=8) as xsbpool:

            wqh_t = [cpool.tile([128, KT, HD], F8, tag=f"wqh{m}", name=f"wqh{m}")
                     for m in range(HL)]
            wql_t = [cpool.tile([128, KT, HD], F8, tag=f"wql{m}", name=f"wql{m}")
                     for m in range(HL)]
            wkh_t = cpool.tile([128, KT, HD], F8, tag="wkh")
            wkl_t = cpool.tile([128, KT, HD], F8, tag="wkl")
            wvh_t = cpool.tile([128, KT, HD], F8, tag="wvh")
            wvl_t = cpool.tile([128, KT, HD], F8, tag="wvl")
            woh_sb = cpool.tile([128, HL, DIM], F8, tag="woh")
            wol_sb = cpool.tile([128, HL, DIM], F8, tag="wol")
            cq_sb = cpool.tile([128, T], BF16, tag="cq")
            sq_sb = cpool.tile([128, T], BF16, tag="sq")
            ck_sb = cpool.tile([128, T], BF16, tag="ck")
            sk_sb = cpool.tile([128, T], BF16, tag="sk")
            maskT_sb = cpool.tile([128, 128], F32, tag="maskT")
            qt_rot = cpool.tile([128, HL, T], BF16, tag="qt")
            kt_rot = cpool.tile([128, T], BF16, tag="kt")
            v_sb = cpool.tile([128, TT, HD], BF16, tag="v")
            aoh = cpool.tile([128, HL, T], F8, tag="aoh")
            aol = cpool.tile([128, HL, T], F8, tag="aol")

            # startup-critical DMA first: K-projection weights
            nc.sync.dma_start(wkh_t[:, 0:8, :], wkh_d.ap()[:, 0:8, :])
            nc.sync.dma_start(wkh_t[:, 8:KT, :], wkh_d.ap()[:, 8:KT, :])
            nc.sync.dma_start(wkl_t, wkl_d.ap())

            def emit_hs_dmas(c, lo=True):
                hsh_sb = hpool.tile([128, KT, CH], F8, tag="hsh", name="hsh_sb")
                hsl_sb = hpool.tile([128, KT, CH], F8, tag="hsl", name="hsl_sb")
                for g in range(4):
                    nc.sync.dma_start(hsh_sb[:, g * 8:(g + 1) * 8, :],
                                      hsh_d.ap()[:, c, g * 8:(g + 1) * 8, :])
                if lo:
                    for g in range(4):
                        nc.sync.dma_start(hsl_sb[:, g * 8:(g + 1) * 8, :],
                                          hsl_d.ap()[:, c, g * 8:(g + 1) * 8, :])
                return hsh_sb, hsl_sb

            def late_consts(hsl_sb):
                # strictly ordered by first use under the term-staged chunk-0
                # emission: V terms, then K-hl/V-lh (hsl), then Q0, ropes, Q1+
                nc.sync.dma_start(wvh_t, wvh_d.ap())
                nc.sync.dma_start(wvl_t, wvl_d.ap())
                nc.sync.dma_start(wqh_t[0], wqh_d.ap()[:, 0])
                nc.sync.dma_start(wql_t[0], wql_d.ap()[:, 0])
                nc.sync.dma_start(cq_sb, cos_q.ap())
                nc.sync.dma_start(sq_sb, sin_q.ap())
                nc.sync.dma_start(maskT_sb, maskT.ap())
                for g in range(4):
                    nc.sync.dma_start(hsl_sb[:, g * 8:(g + 1) * 8, :],
                                      hsl_d.ap()[:, 0, g * 8:(g + 1) * 8, :])
                nc.sync.dma_start(ck_sb, cos_k.ap())
                nc.sync.dma_start(sk_sb, sin_k.ap())
                nc.sync.dma_start(wqh_t[1], wqh_d.ap()[:, 1])
                nc.sync.dma_start(wql_t[1], wql_d.ap()[:, 1])
                for m in range(2, HL):
                    nc.sync.dma_start(wqh_t[m], wqh_d.ap()[:, m])
                    nc.sync.dma_start(wql_t[m], wql_d.ap()[:, m])

            def rope(ps, out_ap, cos_ap, sin_ap):
                """out = ps*cos + halfswap(ps)*sin  (signs baked into sin)."""
                t1 = rpool.tile([128, CH], F32, tag="r1", name="t1")
                t2 = rpool.tile([128, CH], F32, tag="r2", name="t2")
                nc.vector.tensor_mul(t1, ps, cos_ap)
                nc.vector.tensor_mul(t2[0:64, :], ps[64:128, :], sin_ap[0:64, :])
                nc.vector.tensor_mul(t2[64:128, :], ps[0:64, :], sin_ap[64:128, :])
                nc.vector.tensor_add(out_ap, t1, t2)

            def mm3(ps, st_h, st_l, mv_h, mv_l):
                """3-term fp8 DoubleRow accumulation over all KT k-tiles."""
                for kp in range(KP):
                    nc.tensor.matmul(ps, st_h(kp), mv_h(kp),
                                     start=(kp == 0), stop=False, perf_mode=DR)
                for kp in range(KP):
                    nc.tensor.matmul(ps, st_l(kp), mv_h(kp),
                                     start=False, stop=False, perf_mode=DR)
                for kp in range(KP):
                    nc.tensor.matmul(ps, st_h(kp), mv_l(kp),
                                     start=False, stop=(kp == KP - 1),
                                     perf_mode=DR)

            # --- attention group machinery (transposed-scores scheme) ---
            pending = [None]

            def epilogue(st):
                rs, ot, h, q0 = st
                with nc.allow_low_precision("softmax rowsum recip in bf16"):
                    nc.vector.reciprocal(rs, rs)
                t = taopool.tile([128, QB], F32, tag="tao", name="tao")
                nc.vector.tensor_mul(t, ot, rs)
                nc.scalar.copy(aoh[:, h, q0:q0 + QB], t)
                nc.vector.tensor_sub(aol[:, h, q0:q0 + QB], t,
                                     aoh[:, h, q0:q0 + QB])

            def group_units(b, h, qb):
                """Generator: one yield per consumed score k-tile, so group
                work (Act-heavy exp) can be interleaved into PE-heavy Q/O
                projection streams."""
                q0 = b * S + qb * QB
                n_kt = (qb + 1) * (QB // 128)
                # esum accumulates sum_kt et_kt elementwise on DVE (bf16, 2x
                # mode); the final GpSimd partition_all_reduce turns it into
                # softmax rowsums broadcast across partitions. Keeps the
                # rowsum off the PE; bf16 accumulation costs ~0.5% on rs,
                # well inside the error budget.
                esum = espool.tile([128, QB], BF16, tag="esum", name="esum")
                ot = otpool.tile([128, QB], F32, tag="ot", name="ot")
                ets = [None] * n_kt

                def emit_sc(kt):
                    c0 = max(0, kt - qb * (QB // 128)) * 128
                    sc = wpool.tile([128, QB], F32, tag="work", name="sc")
                    nc.tensor.matmul(
                        sc[:, c0:],
                        kt_rot[:, b * S + kt * 128:b * S + (kt + 1) * 128],
                        qt_rot[:, h, q0 + c0:q0 + QB],
                        start=True, stop=True)
                    jd = kt - qb * (QB // 128)
                    if 0 <= jd < QB // 128:
                        nc.vector.tensor_add(sc[:, jd * 128:(jd + 1) * 128],
                                             sc[:, jd * 128:(jd + 1) * 128],
                                             maskT_sb)
                    et = epool.tile([128, QB], BF16, tag="et", name="et")
                    nc.scalar.activation(et[:, c0:], sc[:, c0:], Exp,
                                         bias=0.0, scale=1.0)
                    ets[kt] = (et, c0)

                def consume(kt):
                    et, c0 = ets[kt]
                    if kt == 0:
                        nc.vector.tensor_copy(esum, et)
                    else:
                        nc.vector.tensor_add(esum[:, c0:], esum[:, c0:],
                                             et[:, c0:])
                    nc.tensor.matmul(ot[:, c0:], v_sb[:, b * (S // 128) + kt, :],
                                     et[:, c0:], start=(kt == 0),
                                     stop=(kt == n_kt - 1))
                    ets[kt] = None
                    if kt == 0 and pending[0] is not None:
                        epilogue(pending[0])
                        pending[0] = None

                for kt in range(n_kt):
                    emit_sc(kt)
                    if kt >= 2:
                        consume(kt - 2)
                        yield
                for kt in range(max(0, n_kt - 2), n_kt):
                    consume(kt)
                    yield
                nc.gpsimd.partition_all_reduce(esum, esum, 128,
                                               bass_isa.ReduceOp.add)
                pending[0] = (esum, ot, h, q0)

            def oproj_units(c, direct_out=False):
                """Generator: one yield per O-projection psum tile (fp8
                DoubleRow) for chunk c's 4 token tiles."""
                for tt in range(c * 4, c * 4 + 4):
                    ts = tt * 128
                    for n0 in range(0, DIM, 512):
                        ps = wpool.tile([128, 512], F32, tag="work", name="ps_o")
                        # hp-outer order: the head-pair (0,1) terms of each
                        # tile run before any (2,3) term, covering the last
                        # group's epilogue-chain latency with real work
                        for hp in range(2):
                            h2 = 2 * hp
                            nc.tensor.matmul(
                                ps, aoh[:, h2:h2 + 2, ts:ts + 128],
                                woh_sb[:, h2:h2 + 2, n0:n0 + 512],
                                start=(hp == 0), stop=False, perf_mode=DR)
                            nc.tensor.matmul(
                                ps, aol[:, h2:h2 + 2, ts:ts + 128],
                                woh_sb[:, h2:h2 + 2, n0:n0 + 512],
                                start=False, stop=False, perf_mode=DR)
                            nc.tensor.matmul(
                                ps, aoh[:, h2:h2 + 2, ts:ts + 128],
                                wol_sb[:, h2:h2 + 2, n0:n0 + 512],
                                start=False, stop=(hp == 1), perf_mode=DR)
                        osb = xsbpool.tile([128, 512], BF16, tag="osb",
                                           name="osb")
                        if (tt * 8 + n0 // 512) % 2 == 0:
                            nc.scalar.copy(osb, ps)
                        else:
                            nc.vector.tensor_copy(osb, ps)
                        nc.sync.dma_start(out.ap()[ts:ts + 128, n0:n0 + 512],
                                          osb)
                        yield

            def drain(gen, n=10 ** 9):
                """Pull up to n units; True if the generator is exhausted."""
                for _ in range(n):
                    if next(gen, _SENTINEL) is _SENTINEL:
                        return True
                return False

            _SENTINEL = object()

            def hs_dma_closures(c):
                """Allocate next chunk's hs tiles; return deferred DMA
                emitters so the transfers can be paced into the O-proj
                stream (fair-sharing the DMA engines with osb writes)."""
                hsh_sb = hpool.tile([128, KT, CH], F8, tag="hsh", name="hsh_sb")
                hsl_sb = hpool.tile([128, KT, CH], F8, tag="hsl", name="hsl_sb")

                def mk(dst, src, g):
                    return lambda: nc.sync.dma_start(
                        dst[:, g * 8:(g + 1) * 8, :],
                        src[:, c, g * 8:(g + 1) * 8, :])

                fs = [mk(hsh_sb, hsh_d.ap(), g) for g in range(4)]
                fs += [mk(hsl_sb, hsl_d.ap(), g) for g in range(4)]
                return (hsh_sb, hsl_sb), fs

            # --- main schedule ---
            hs_cur = emit_hs_dmas(0, lo=False)
            for c in range(NCHUNK):
                hsh_sb, hsl_sb = hs_cur
                t0 = c * CH
                b, qb = c // 2, c % 2
                def st(w):
                    return lambda kp: w[:, 2 * kp:2 * kp + 2, :]

                def mv(x):
                    return lambda kp: x[:, 2 * kp:2 * kp + 2, :]

                def mm(ps, s, v, start=False, stop=False):
                    nc.tensor.matmul(ps, s, v, start=start, stop=stop,
                                     perf_mode=DR)

                if c == 0:
                    # Term-staged startup, emission ordered to match DMA
                    # arrival: K-hh/lh (wk+hsh), V-hh/hl (wv), K-hl & V-lh
                    # (hsl), then Q0 (wq0), so the PE is never waiting on a
                    # transfer that sits behind unneeded bytes.
                    kh, kl = st(wkh_t), st(wkl_t)
                    vh, vl = st(wvh_t), st(wvl_t)
                    qh, ql = st(wqh_t[0]), st(wql_t[0])
                    mh, ml = mv(hsh_sb), mv(hsl_sb)
                    psK = wpool.tile([128, CH], F32, tag="work", name="ps_k")
                    for kp in range(KP):
                        mm(psK, kh(kp), mh(kp), start=(kp == 0))
                    late_consts(hsl_sb)
                    for kp in range(KP):
                        mm(psK, kl(kp), mh(kp))
                    psV = []
                    for vi in range(CH // 128):
                        v0 = vi * 128
                        pv = wpool.tile([128, HD], F32, tag="work",
                                        name="ps_v")
                        for kp in range(KP):
                            mm(pv, hsh_sb[:, 2 * kp:2 * kp + 2, v0:v0 + 128],
                               vh(kp), start=(kp == 0))
                        for kp in range(KP):
                            mm(pv, hsh_sb[:, 2 * kp:2 * kp + 2, v0:v0 + 128],
                               vl(kp))
                        psV.append(pv)
                    psQ = wpool.tile([128, CH], F32, tag="work", name="ps_q")
                    for kp in range(KP):
                        mm(psQ, qh(kp), mh(kp), start=(kp == 0))
                    for kp in range(KP):
                        mm(psQ, ql(kp), mh(kp))
                    # stage B: hsl-dependent third terms
                    for kp in range(KP):
                        mm(psK, kh(kp), ml(kp), stop=(kp == KP - 1))
                    rope(psK, kt_rot[:, t0:t0 + CH],
                         ck_sb[:, t0:t0 + CH], sk_sb[:, t0:t0 + CH])
                    for vi in range(CH // 128):
                        v0 = vi * 128
                        for kp in range(KP):
                            mm(psV[vi],
                               hsl_sb[:, 2 * kp:2 * kp + 2, v0:v0 + 128],
                               vh(kp), stop=(kp == KP - 1))
                        nc.scalar.copy(v_sb[:, t0 // 128 + vi, :], psV[vi])
                    for kp in range(KP):
                        mm(psQ, qh(kp), ml(kp), stop=(kp == KP - 1))
                    rope(psQ, qt_rot[:, 0, t0:t0 + CH],
                         cq_sb[:, t0:t0 + CH], sq_sb[:, t0:t0 + CH])
                    m_start = 1
                    active = None
                    created = 0
                else:
                    # K projection first (its rope unblocks all groups)
                    ps = wpool.tile([128, CH], F32, tag="work", name="ps_k")
                    mm3(ps, st(wkh_t), st(wkl_t), mv(hsh_sb), mv(hsl_sb))
                    rope(ps, kt_rot[:, t0:t0 + CH],
                         ck_sb[:, t0:t0 + CH], sk_sb[:, t0:t0 + CH])
                    # V projection
                    for vi in range(CH // 128):
                        tt = t0 // 128 + vi
                        ps = wpool.tile([128, HD], F32, tag="work",
                                        name="ps_v")
                        v0 = vi * 128
                        mm3(ps,
                            lambda kp: hsh_sb[:, 2 * kp:2 * kp + 2,
                                              v0:v0 + 128],
                            lambda kp: hsl_sb[:, 2 * kp:2 * kp + 2,
                                              v0:v0 + 128],
                            st(wvh_t), st(wvl_t))
                        nc.scalar.copy(v_sb[:, tt, :], ps)
                    m_start = 0
                    active = None
                    created = 0
                # Q heads in quarter-batches; group h-1's units interleave
                # into head h's matmul stream (PE-heavy, Act-light)
                for m in range(m_start, HL):
                    ps = wpool.tile([128, CH], F32, tag="work", name="ps_q")
                    sh, sl = st(wqh_t[m]), st(wql_t[m])
                    mh, ml = mv(hsh_sb), mv(hsl_sb)
                    nb = 8 if qb else 4
                    for bi in range(nb):
                        k0 = bi * KP // nb
                        k1 = (bi + 1) * KP // nb
                        for kp in range(k0, k1):
                            nc.tensor.matmul(ps, sh(kp), mh(kp),
                                             start=(kp == 0), stop=False,
                                             perf_mode=DR)
                        for kp in range(k0, k1):
                            nc.tensor.matmul(ps, sl(kp), mh(kp),
                                             start=False, stop=False,
                                             perf_mode=DR)
                        for kp in range(k0, k1):
                            nc.tensor.matmul(ps, sh(kp), ml(kp), start=False,
                                             stop=(bi == nb - 1
                                                   and kp == k1 - 1),
                                             perf_mode=DR)
                        if active is not None:
                            drain(active, 1)
                    rope(ps, qt_rot[:, m, t0:t0 + CH],
                         cq_sb[:, t0:t0 + CH], sq_sb[:, t0:t0 + CH])
                    if created < m:
                        if active is not None:
                            drain(active)
                        active = group_units(b, created, qb)
                        created += 1
                # remaining groups (h=2,3) interleave with the O projection
                # of the previous chunk; chunk 0 has no O-proj to interleave.
                # Next-chunk hs DMAs (and wo, at c==0) pace into the stream.
                tail_gens = [active, group_units(b, HL - 1, qb)]
                active = None
                feed = []
                if c + 1 < NCHUNK:
                    hs_cur, feed = hs_dma_closures(c + 1)
                if c == 0:
                    feed.append(lambda: nc.sync.dma_start(woh_sb, woh_d.ap()))
                    feed.append(lambda: nc.sync.dma_start(wol_sb, wol_d.ap()))
                if c >= 1:
                    op = oproj_units(c - 1)
                    gi = 0
                    done_op = False
                    opn = 0
                    while not done_op:
                        if gi < len(tail_gens):
                            if drain(tail_gens[gi], 1):
                                gi += 1
                                continue
                        done_op = drain(op, 2 if qb else 4)
                        opn += 1
                        if feed and opn % 2 == 0:
                            feed.pop(0)()
                    for g in tail_gens[gi:]:
                        drain(g)
                else:
                    for f in feed:
                        f()
                    feed = []
                    for g in tail_gens:
                        drain(g)
                for f in feed:
                    f()
                if c == NCHUNK - 1 and pending[0] is not None:
                    # flush the last epilogue now: its DVE/Act ops run while
                    # the PE works through the final O projection below
                    epilogue(pending[0])
                    pending[0] = None
            drain(oproj_units(NCHUNK - 1))
    nc.compile()
    return nc


def _get_nc():
    if "nc" not in _CACHE:
        _CACHE["nc"] = _build()
    return _CACHE["nc"]


def _split8(x: np.ndarray):
    """Split f32 array into (hi, lo) e4m3 pair with hi + lo ~= x."""
    hi = x.astype(E4M3)
    lo = (x - hi.astype(np.float32)).astype(E4M3)
    return hi, lo


def _prep_inputs(inputs) -> list[dict]:
    bf16 = ml_dtypes.bfloat16
    hs = np.asarray(inputs["hidden_states"], dtype=np.float32).reshape(T, DIM)
    hsT = np.ascontiguousarray(hs.T)
    hsh, hsl = _split8(hsT)

    def swz_hs(x):  # [DIM, T] -> [128, NCHUNK, KT, CH] (SBUF layout)
        return np.ascontiguousarray(
            x.reshape(KT, 128, NCHUNK, CH).transpose(1, 2, 0, 3))

    hsh = swz_hs(hsh)
    hsl = swz_hs(hsl)

    fc = np.asarray(inputs["freqs_cos"], dtype=np.float32).reshape(T, HD // 2).T
    fs = np.asarray(inputs["freqs_sin"], dtype=np.float32).reshape(T, HD // 2).T
    cos2 = np.concatenate([fc, fc], axis=0)            # [128, T]
    sin2 = np.concatenate([-fs, fs], axis=0)           # signed half-rotation
    cos_qv = np.ascontiguousarray(cos2 * (SCALE / S_Q)).astype(bf16)
    sin_qv = np.ascontiguousarray(sin2 * (SCALE / S_Q)).astype(bf16)
    cos_kv = np.ascontiguousarray(cos2 * (1.0 / S_K)).astype(bf16)
    sin_kv = np.ascontiguousarray(sin2 * (1.0 / S_K)).astype(bf16)

    maskT = np.ascontiguousarray(
        np.asarray(inputs["attention_mask"], dtype=np.float32)[0, 0, :128, :128].T)

    perm = np.concatenate([np.arange(0, HD, 2), np.arange(1, HD, 2)])
    Wq = np.asarray(inputs["Wq"], dtype=np.float32)
    Wk = np.asarray(inputs["Wk"], dtype=np.float32)
    Wv = np.asarray(inputs["Wv"], dtype=np.float32)
    Wo = np.asarray(inputs["Wo"], dtype=np.float32)

    def swz_w(x, nh):  # [DIM, nh*HD] -> [128, nh, KT, HD]
        return np.ascontiguousarray(
            x.reshape(KT, 128, nh, HD).transpose(1, 2, 0, 3))

    in_maps = []
    for c in range(N_CORES):
        wq_c = np.concatenate(
            [Wq[:, (c * HL + h) * HD:(c * HL + h + 1) * HD][:, perm]
             for h in range(HL)], axis=1) * S_Q
        wk_c = Wk[:, c * HD:(c + 1) * HD][:, perm] * S_K
        wv_c = Wv[:, c * HD:(c + 1) * HD] * S_V
        wo_c = Wo[c * HL * HD:(c + 1) * HL * HD, :] * S_O
        wqh, wql = _split8(wq_c)
        wkh, wkl = _split8(wk_c)
        wvh, wvl = _split8(wv_c)
        woh, wol = _split8(wo_c)
        in_maps.append({
            "hsh": hsh, "hsl": hsl,
            "wqh": swz_w(wqh, HL), "wql": swz_w(wql, HL),
            "wkh": swz_w(wkh, 1).reshape(128, KT, HD),
            "wkl": swz_w(wkl, 1).reshape(128, KT, HD),
            "wvh": swz_w(wvh, 1).reshape(128, KT, HD),
            "wvl": swz_w(wvl, 1).reshape(128, KT, HD),
            "woh": np.ascontiguousarray(
                woh.reshape(HL, 128, DIM).transpose(1, 0, 2)),
            "wol": np.ascontiguousarray(
                wol.reshape(HL, 128, DIM).transpose(1, 0, 2)),
            "cos_q": cos_qv, "sin_q": sin_qv,
            "cos_k": cos_kv, "sin_k": sin_kv,
            "maskT": maskT,
        })
    return in_maps


def kernel(**inputs) -> np.ndarray:
    nc = _get_nc()
    in_maps = _prep_inputs(inputs)
    res = bass_utils.run_bass_kernel_spmd(nc, in_maps,
                                          core_ids=list(range(N_CORES)))
    acc = np.zeros((T, DIM), dtype=np.float32)
    for c in range(N_CORES):
        acc += np.asarray(res.results[c]["out"], dtype=np.float32)
    return (acc * (1.0 / (S_V * S_O))).reshape(B, S, DIM)


# revision 73
# speedup vs baseline: 1.3448x; 1.0308x over previous
"""Trainium2 Bass kernel for MllamaTextSdpaAttention (GQA + RoPE + causal SDPA).

Strategy: tensor-parallel over heads across 8 NeuronCores. Core c owns
q-heads [4c, 4c+4) and kv-head c (kv groups intact). Each core computes
hidden @ Wq/Wk/Wv slices, RoPE, causal attention for its heads, and its
row-slice of the Wo matmul, yielding a partial [T, DIM] output (bf16).
The host sums the 8 partials in f32.

Key techniques:
- All four projections (Q/K/V/O) run on the PE in fp8e4m3 DoubleRow mode
  (2 k-tiles of contraction per instruction at 0.5 cycles/column = 4x the
  bf16 FLOP rate). Accuracy is preserved with a 3-term residual split:
  each operand X is split (host-side for inputs/weights, on-device for
  ao) into Xh = fp8(X), Xl = fp8(X - Xh), and W@X ~= Wh@Xh + Wl@Xh +
  Wh@Xl. Net cost: 0.75x the bf16 column count. Weights are pre-scaled
  (x32 Wq/Wk/Wo, x16 Wv) into e4m3's normal range; descales fold into
  the RoPE tables and the host-side gather.
- All inputs are pre-swizzled on the host into exact SBUF layouts so
  every DMA moves >=512-byte contiguous runs (full 360 GB/s; under 512B
  the DMA engines run at half rate).
- Attention stays bf16 on the PE (scores + P@V only): transposed scores
  (scT = K_rot^T.T @ Q_rot^T), exp on Act feeds P@V directly. The
  softmax rowsums are computed OFF the PE: et tiles are summed
  elementwise on DVE (bf16, 2x/4x modes), then one GpSimd
  partition_all_reduce broadcasts the rowsum to all partitions; the
  reciprocal+normalize epilogue is deferred one group so the PE never
  waits on it. RoPE as a half-rotation with host-permuted weight
  columns. Causality at 128-block granularity.
- Schedule: per chunk, K and V run first (term-staged against DMA
  arrival for chunk 0), then Q heads emitted in sub-batches with the
  previous head's attention-group units interleaved into the stream
  (group work is Act-heavy, projections are PE-heavy). The O projection
  for chunk c-1 interleaves with chunk c's last two groups and paces
  the next chunk's hs DMAs, spreading output DMA across the kernel.
  Only chunk 3's O-proj trails the last attention group, with its
  epilogue chain hidden under chunk 2's O-proj.
"""

import numpy as np
import ml_dtypes

import concourse.bacc as bacc
import concourse.bass as bass
import concourse.bass_isa as bass_isa
import concourse.mybir as mybir
from concourse.tile import TileContext
from concourse import bass_utils

BF16 = mybir.dt.bfloat16
F32 = mybir.dt.float32
F8 = mybir.dt.float8e4
E4M3 = ml_dtypes.float8_e4m3

B, S, DIM = 2, 1024, 4096
T = B * S                     # 2048 tokens, batch-major
N_HEADS, N_KV = 32, 8
HD = 128                      # head dim == partition count
N_CORES = 8
HL = N_HEADS // N_CORES       # 4 local q-heads per core
KT = DIM // 128               # 32 feature tiles
KP = KT // 2                  # 16 k-tile PAIRS (DoubleRow)
CH = 512                      # projection token-chunk
NCHUNK = T // CH
QB = 512                      # attention q-block width
TT = T // 128                 # 16 token tiles global
SCALE = 1.0 / float(np.sqrt(HD))
S_Q = 32.0                    # weight pre-scales for fp8 range
S_K = 32.0
S_V = 16.0
S_O = 32.0
DR = mybir.MatmulPerfMode.DoubleRow

_CACHE: dict = {}


def _build():
    nc = bacc.Bacc("TRN2", target_bir_lowering=False, debug=False,
                   enable_asserts=False, dynamic_dma_scratch_size=2048)

    # all tensors pre-swizzled host-side into SBUF layout (partition-major)
    hsh_d = nc.dram_tensor("hsh", [128, NCHUNK, KT, CH], F8, kind="ExternalInput")
    hsl_d = nc.dram_tensor("hsl", [128, NCHUNK, KT, CH], F8, kind="ExternalInput")
    wqh_d = nc.dram_tensor("wqh", [128, HL, KT, HD], F8, kind="ExternalInput")
    wql_d = nc.dram_tensor("wql", [128, HL, KT, HD], F8, kind="ExternalInput")
    wkh_d = nc.dram_tensor("wkh", [128, KT, HD], F8, kind="ExternalInput")
    wkl_d = nc.dram_tensor("wkl", [128, KT, HD], F8, kind="ExternalInput")
    wvh_d = nc.dram_tensor("wvh", [128, KT, HD], F8, kind="ExternalInput")
    wvl_d = nc.dram_tensor("wvl", [128, KT, HD], F8, kind="ExternalInput")
    woh_d = nc.dram_tensor("woh", [128, HL, DIM], F8, kind="ExternalInput")
    wol_d = nc.dram_tensor("wol", [128, HL, DIM], F8, kind="ExternalInput")
    cos_q = nc.dram_tensor("cos_q", [HD, T], BF16, kind="ExternalInput")
    sin_q = nc.dram_tensor("sin_q", [HD, T], BF16, kind="ExternalInput")
    cos_k = nc.dram_tensor("cos_k", [HD, T], BF16, kind="ExternalInput")
    sin_k = nc.dram_tensor("sin_k", [HD, T], BF16, kind="ExternalInput")
    maskT = nc.dram_tensor("maskT", [128, 128], F32, kind="ExternalInput")
    out = nc.dram_tensor("out", [T, DIM], BF16, kind="ExternalOutput")

    Exp = mybir.ActivationFunctionType.Exp

    with TileContext(nc) as tc:
        with tc.tile_pool(name="consts", bufs=1) as cpool, \
             tc.tile_pool(name="hs", bufs=2) as hpool, \
             tc.tile_pool(name="rope_tmp", bufs=1) as rpool, \
             tc.tile_pool(name="work_ps", bufs=6, space=bass.MemorySpace.PSUM) as wpool, \
             tc.tile_pool(name="ot_ps", bufs=2, space=bass.MemorySpace.PSUM) as otpool, \
             tc.tile_pool(name="et", bufs=4) as epool, \
             tc.tile_pool(name="esum", bufs=1) as espool, \
             tc.tile_pool(name="tao", bufs=1) as taopool, \
             tc.tile_pool(name="out_sb", bufs=4) as xsbpool:

            wqh_t = [cpool.tile([128, KT, HD], F8, tag=f"wqh{m}", name=f"wqh{m}")
                     for m in range(HL)]
            wql_t = [cpool.tile([128, KT, HD], F8, tag=f"wql{m}", name=f"wql{m}")
                     for m in range(HL)]
            wkh_t = cpool.tile([128, KT, HD], F8, tag="wkh")
            wkl_t = cpool.tile([128, KT, HD], F8, tag="wkl")
            wvh_t = cpool.tile([128, KT, HD], F8, tag="wvh")
            wvl_t = cpool.tile([128, KT, HD], F8, tag="wvl")
            woh_sb = cpool.tile([128, HL, DIM], F8, tag="woh")
            wol_sb = cpool.tile([128, HL, DIM], F8, tag="wol")
            cq_sb = cpool.tile([128, T], BF16, tag="cq")
            sq_sb = cpool.tile([128, T], BF16, tag="sq")
            ck_sb = cpool.tile([128, T], BF16, tag="ck")
            sk_sb = cpool.tile([128, T], BF16, tag="sk")
            maskT_sb = cpool.tile([128, 128], F32, tag="maskT")
            qt_rot = cpool.tile([128, HL, T], BF16, tag="qt")
            kt_rot = cpool.tile([128, T], BF16, tag="kt")
            v_sb = cpool.tile([128, TT, HD], BF16, tag="v")
            aoh = cpool.tile([128, HL, T], F8, tag="aoh")
            aol = cpool.tile([128, HL, T], F8, tag="aol")

            # startup-critical DMA first: K-projection weights
            nc.sync.dma_start(wkh_t[:, 0:8, :], wkh_d.ap()[:, 0:8, :])
            nc.sync.dma_start(wkh_t[:, 8:KT, :], wkh_d.ap()[:, 8:KT, :])
            nc.sync.dma_start(wkl_t, wkl_d.ap())

            def emit_hs_dmas(c, lo=True):
                hsh_sb = hpool.tile([128, KT, CH], F8, tag="hsh", name="hsh_sb")
                hsl_sb = hpool.tile([128, KT, CH], F8, tag="hsl", name="hsl_sb")
                for g in range(4):
                    nc.sync.dma_start(hsh_sb[:, g * 8:(g + 1) * 8, :],
                                      hsh_d.ap()[:, c, g * 8:(g + 1) * 8, :])
                if lo:
                    for g in range(4):
                        nc.sync.dma_start(hsl_sb[:, g * 8:(g + 1) * 8, :],
                                          hsl_d.ap()[:, c, g * 8:(g + 1) * 8, :])
                return hsh_sb, hsl_sb

            def late_consts(hsl_sb):
                # strictly ordered by first use under the term-staged chunk-0
                # emission: V terms, then K-hl/V-lh (hsl), then Q0, ropes, Q1+
                nc.sync.dma_start(wvh_t, wvh_d.ap())
                nc.sync.dma_start(wvl_t, wvl_d.ap())
                nc.sync.dma_start(wqh_t[0], wqh_d.ap()[:, 0])
                nc.sync.dma_start(wql_t[0], wql_d.ap()[:, 0])
                # cos/sin tables: chunk 0 only needs its own 512-col slices
                # now; the rest ride along with later chunks' hs feeds
                nc.sync.dma_start(cq_sb[:, 0:CH], cos_q.ap()[:, 0:CH])
                nc.sync.dma_start(sq_sb[:, 0:CH], sin_q.ap()[:, 0:CH])
                nc.sync.dma_start(maskT_sb, maskT.ap())
                for g in range(4):
                    nc.sync.dma_start(hsl_sb[:, g * 8:(g + 1) * 8, :],
                                      hsl_d.ap()[:, 0, g * 8:(g + 1) * 8, :])
                nc.sync.dma_start(ck_sb[:, 0:CH], cos_k.ap()[:, 0:CH])
                nc.sync.dma_start(sk_sb[:, 0:CH], sin_k.ap()[:, 0:CH])
                nc.sync.dma_start(wqh_t[1], wqh_d.ap()[:, 1])
                nc.sync.dma_start(wql_t[1], wql_d.ap()[:, 1])
                for m in range(2, HL):
                    nc.sync.dma_start(wqh_t[m], wqh_d.ap()[:, m])
                    nc.sync.dma_start(wql_t[m], wql_d.ap()[:, m])

            def rope(ps, out_ap, cos_ap, sin_ap):
                """out = ps*cos + halfswap(ps)*sin  (signs baked into sin)."""
                t1 = rpool.tile([128, CH], F32, tag="r1", name="t1")
                t2 = rpool.tile([128, CH], F32, tag="r2", name="t2")
                nc.vector.tensor_mul(t1, ps, cos_ap)
                nc.vector.tensor_mul(t2[0:64, :], ps[64:128, :], sin_ap[0:64, :])
                nc.vector.tensor_mul(t2[64:128, :], ps[0:64, :], sin_ap[64:128, :])
                nc.vector.tensor_add(out_ap, t1, t2)

            def mm3(ps, st_h, st_l, mv_h, mv_l):
                """3-term fp8 DoubleRow accumulation over all KT k-tiles."""
                for kp in range(KP):
                    nc.tensor.matmul(ps, st_h(kp), mv_h(kp),
                                     start=(kp == 0), stop=False, perf_mode=DR)
                for kp in range(KP):
                    nc.tensor.matmul(ps, st_l(kp), mv_h(kp),
                                     start=False, stop=False, perf_mode=DR)
                for kp in range(KP):
                    nc.tensor.matmul(ps, st_h(kp), mv_l(kp),
                                     start=False, stop=(kp == KP - 1),
                                     perf_mode=DR)

            # --- attention group machinery (transposed-scores scheme) ---
            pending = [None]

            def epilogue(st):
                rs, ot, h, q0 = st
                with nc.allow_low_precision("softmax rowsum recip in bf16"):
                    nc.vector.reciprocal(rs, rs)
                t = taopool.tile([128, QB], F32, tag="tao", name="tao")
                nc.vector.tensor_mul(t, ot, rs)
                nc.scalar.copy(aoh[:, h, q0:q0 + QB], t)
                nc.vector.tensor_sub(aol[:, h, q0:q0 + QB], t,
                                     aoh[:, h, q0:q0 + QB])

            def group_units(b, h, qb):
                """Generator: one yield per consumed score k-tile, so group
                work (Act-heavy exp) can be interleaved into PE-heavy Q/O
                projection streams."""
                q0 = b * S + qb * QB
                n_kt = (qb + 1) * (QB // 128)
                # esum accumulates sum_kt et_kt elementwise on DVE (bf16, 2x
                # mode); the final GpSimd partition_all_reduce turns it into
                # softmax rowsums broadcast across partitions. Keeps the
                # rowsum off the PE; bf16 accumulation costs ~0.5% on rs,
                # well inside the error budget.
                esum = espool.tile([128, QB], BF16, tag="esum", name="esum")
                ot = otpool.tile([128, QB], F32, tag="ot", name="ot")
                ets = [None] * n_kt

                def emit_sc(kt):
                    c0 = max(0, kt - qb * (QB // 128)) * 128
                    sc = wpool.tile([128, QB], F32, tag="work", name="sc")
                    nc.tensor.matmul(
                        sc[:, c0:],
                        kt_rot[:, b * S + kt * 128:b * S + (kt + 1) * 128],
                        qt_rot[:, h, q0 + c0:q0 + QB],
                        start=True, stop=True)
                    jd = kt - qb * (QB // 128)
                    if 0 <= jd < QB // 128:
                        nc.vector.tensor_add(sc[:, jd * 128:(jd + 1) * 128],
                                             sc[:, jd * 128:(jd + 1) * 128],
                                             maskT_sb)
                    et = epool.tile([128, QB], BF16, tag="et", name="et")
                    nc.scalar.activation(et[:, c0:], sc[:, c0:], Exp,
                                         bias=0.0, scale=1.0)
                    ets[kt] = (et, c0)

                def consume(kt):
                    et, c0 = ets[kt]
                    if kt == 0:
                        nc.vector.tensor_copy(esum, et)
                    else:
                        nc.vector.tensor_add(esum[:, c0:], esum[:, c0:],
                                             et[:, c0:])
                    nc.tensor.matmul(ot[:, c0:], v_sb[:, b * (S // 128) + kt, :],
                                     et[:, c0:], start=(kt == 0),
                                     stop=(kt == n_kt - 1))
                    ets[kt] = None
                    if kt == 0 and pending[0] is not None:
                        epilogue(pending[0])
                        pending[0] = None

                for kt in range(n_kt):
                    emit_sc(kt)
                    if kt >= 2:
                        consume(kt - 2)
                        yield
                for kt in range(max(0, n_kt - 2), n_kt):
                    consume(kt)
                    yield
                nc.gpsimd.partition_all_reduce(esum, esum, 128,
                                               bass_isa.ReduceOp.add)
                pending[0] = (esum, ot, h, q0)

            def oproj_units(c, direct_out=False):
                """Generator: one yield per O-projection psum tile (fp8
                DoubleRow) for chunk c's 4 token tiles. Two 512-col psum
                tiles pair into one [128,1024] osb buffer and ONE output
                DMA, halving the HWDGE issue pressure (625ns per DMA is
                otherwise at parity with the PE's tile rate)."""
                for tt in range(c * 4, c * 4 + 4):
                    ts = tt * 128
                    for n0 in range(0, DIM, 1024):
                        osb = xsbpool.tile([128, 1024], BF16, tag="osb",
                                           name="osb")
                        on_act = (tt * 4 + n0 // 1024) % 2 == 0
                        for half in range(2):
                            nh = n0 + half * 512
                            ps = wpool.tile([128, 512], F32, tag="work",
                                            name="ps_o")
                            # hp-outer order: the head-pair (0,1) terms run
                            # before any (2,3) term, covering the last
                            # group's epilogue-chain latency with real work
                            for hp in range(2):
                                h2 = 2 * hp
                                nc.tensor.matmul(
                                    ps, aoh[:, h2:h2 + 2, ts:ts + 128],
                                    woh_sb[:, h2:h2 + 2, nh:nh + 512],
                                    start=(hp == 0), stop=False, perf_mode=DR)
                                nc.tensor.matmul(
                                    ps, aol[:, h2:h2 + 2, ts:ts + 128],
                                    woh_sb[:, h2:h2 + 2, nh:nh + 512],
                                    start=False, stop=False, perf_mode=DR)
                                nc.tensor.matmul(
                                    ps, aoh[:, h2:h2 + 2, ts:ts + 128],
                                    wol_sb[:, h2:h2 + 2, nh:nh + 512],
                                    start=False, stop=(hp == 1), perf_mode=DR)
                            dst = osb[:, half * 512:(half + 1) * 512]
                            if on_act:
                                nc.scalar.copy(dst, ps)
                            else:
                                nc.vector.tensor_copy(dst, ps)
                            if half == 0:
                                yield
                        nc.sync.dma_start(
                            out.ap()[ts:ts + 128, n0:n0 + 1024], osb)
                        yield

            def drain(gen, n=10 ** 9):
                """Pull up to n units; True if the generator is exhausted."""
                for _ in range(n):
                    if next(gen, _SENTINEL) is _SENTINEL:
                        return True
                return False

            _SENTINEL = object()

            def hs_dma_closures(c):
                """Allocate next chunk's hs tiles; return deferred DMA
                emitters so the transfers can be paced into the O-proj
                stream (fair-sharing the DMA engines with osb writes)."""
                hsh_sb = hpool.tile([128, KT, CH], F8, tag="hsh", name="hsh_sb")
                hsl_sb = hpool.tile([128, KT, CH], F8, tag="hsl", name="hsl_sb")

                def mk(dst, src, g):
                    return lambda: nc.sync.dma_start(
                        dst[:, g * 8:(g + 1) * 8, :],
                        src[:, c, g * 8:(g + 1) * 8, :])

                def mk_cs(dst, src):
                    return lambda: nc.sync.dma_start(
                        dst[:, c * CH:(c + 1) * CH],
                        src[:, c * CH:(c + 1) * CH])

                fs = [mk(hsh_sb, hsh_d.ap(), g) for g in range(4)]
                fs += [mk_cs(ck_sb, cos_k.ap()), mk_cs(sk_sb, sin_k.ap())]
                fs += [mk(hsl_sb, hsl_d.ap(), g) for g in range(4)]
                fs += [mk_cs(cq_sb, cos_q.ap()), mk_cs(sq_sb, sin_q.ap())]
                return (hsh_sb, hsl_sb), fs

            # --- main schedule ---
            hs_cur = emit_hs_dmas(0, lo=False)
            for c in range(NCHUNK):
                hsh_sb, hsl_sb = hs_cur
                t0 = c * CH
                b, qb = c // 2, c % 2
                def st(w):
                    return lambda kp: w[:, 2 * kp:2 * kp + 2, :]

                def mv(x):
                    return lambda kp: x[:, 2 * kp:2 * kp + 2, :]

                def mm(ps, s, v, start=False, stop=False):
                    nc.tensor.matmul(ps, s, v, start=start, stop=stop,
                                     perf_mode=DR)

                if c == 0:
                    # Term-staged startup, emission ordered to match DMA
                    # arrival: K-hh/lh (wk+hsh), V-hh/hl (wv), K-hl & V-lh
                    # (hsl), then Q0 (wq0), so the PE is never waiting on a
                    # transfer that sits behind unneeded bytes.
                    kh, kl = st(wkh_t), st(wkl_t)
                    vh, vl = st(wvh_t), st(wvl_t)
                    qh, ql = st(wqh_t[0]), st(wql_t[0])
                    mh, ml = mv(hsh_sb), mv(hsl_sb)
                    psK = wpool.tile([128, CH], F32, tag="work", name="ps_k")
                    for kp in range(KP):
                        mm(psK, kh(kp), mh(kp), start=(kp == 0))
                    late_consts(hsl_sb)
                    for kp in range(KP):
                        mm(psK, kl(kp), mh(kp))
                    psV = []
                    for vi in range(CH // 128):
                        v0 = vi * 128
                        pv = wpool.tile([128, HD], F32, tag="work",
                                        name="ps_v")
                        for kp in range(KP):
                            mm(pv, hsh_sb[:, 2 * kp:2 * kp + 2, v0:v0 + 128],
                               vh(kp), start=(kp == 0))
                        for kp in range(KP):
                            mm(pv, hsh_sb[:, 2 * kp:2 * kp + 2, v0:v0 + 128],
                               vl(kp))
                        psV.append(pv)
                    psQ = wpool.tile([128, CH], F32, tag="work", name="ps_q")
                    for kp in range(KP):
                        mm(psQ, qh(kp), mh(kp), start=(kp == 0))
                    for kp in range(KP):
                        mm(psQ, ql(kp), mh(kp))
                    # stage B: hsl-dependent third terms
                    for kp in range(KP):
                        mm(psK, kh(kp), ml(kp), stop=(kp == KP - 1))
                    rope(psK, kt_rot[:, t0:t0 + CH],
                         ck_sb[:, t0:t0 + CH], sk_sb[:, t0:t0 + CH])
                    for vi in range(CH // 128):
                        v0 = vi * 128
                        for kp in range(KP):
                            mm(psV[vi],
                               hsl_sb[:, 2 * kp:2 * kp + 2, v0:v0 + 128],
                               vh(kp), stop=(kp == KP - 1))
                        nc.scalar.copy(v_sb[:, t0 // 128 + vi, :], psV[vi])
                    for kp in range(KP):
                        mm(psQ, qh(kp), ml(kp), stop=(kp == KP - 1))
                    rope(psQ, qt_rot[:, 0, t0:t0 + CH],
                         cq_sb[:, t0:t0 + CH], sq_sb[:, t0:t0 + CH])
                    m_start = 1
                    active = None
                    created = 0
                else:
                    # K projection first (its rope unblocks all groups)
                    ps = wpool.tile([128, CH], F32, tag="work", name="ps_k")
                    mm3(ps, st(wkh_t), st(wkl_t), mv(hsh_sb), mv(hsl_sb))
                    rope(ps, kt_rot[:, t0:t0 + CH],
                         ck_sb[:, t0:t0 + CH], sk_sb[:, t0:t0 + CH])
                    # V projection
                    for vi in range(CH // 128):
                        tt = t0 // 128 + vi
                        ps = wpool.tile([128, HD], F32, tag="work",
                                        name="ps_v")
                        v0 = vi * 128
                        mm3(ps,
                            lambda kp: hsh_sb[:, 2 * kp:2 * kp + 2,
                                              v0:v0 + 128],
                            lambda kp: hsl_sb[:, 2 * kp:2 * kp + 2,
                                              v0:v0 + 128],
                            st(wvh_t), st(wvl_t))
                        nc.scalar.copy(v_sb[:, tt, :], ps)
                    m_start = 0
                    active = None
                    created = 0
                # Q heads in quarter-batches; group h-1's units interleave
                # into head h's matmul stream (PE-heavy, Act-light)
                for m in range(m_start, HL):
                    ps = wpool.tile([128, CH], F32, tag="work", name="ps_q")
                    sh, sl = st(wqh_t[m]), st(wql_t[m])
                    mh, ml = mv(hsh_sb), mv(hsl_sb)
                    nb = 8 if qb else 4
                    for bi in range(nb):
                        k0 = bi * KP // nb
                        k1 = (bi + 1) * KP // nb
                        for kp in range(k0, k1):
                            nc.tensor.matmul(ps, sh(kp), mh(kp),
                                             start=(kp == 0), stop=False,
                                             perf_mode=DR)
                        for kp in range(k0, k1):
                            nc.tensor.matmul(ps, sl(kp), mh(kp),
                                             start=False, stop=False,
                                             perf_mode=DR)
                        for kp in range(k0, k1):
                            nc.tensor.matmul(ps, sh(kp), ml(kp), start=False,
                                             stop=(bi == nb - 1
                                                   and kp == k1 - 1),
                                             perf_mode=DR)
                        if active is not None:
                            drain(active, 1)
                    rope(ps, qt_rot[:, m, t0:t0 + CH],
                         cq_sb[:, t0:t0 + CH], sq_sb[:, t0:t0 + CH])
                    if created < m:
                        if active is not None:
                            drain(active)
                        active = group_units(b, created, qb)
                        created += 1
                # remaining groups (h=2,3) interleave with the O projection
                # of the previous chunk; chunk 0 has no O-proj to interleave.
                # Next-chunk hs DMAs (and wo, at c==0) pace into the stream.
                tail_gens = [active, group_units(b, HL - 1, qb)]
                active = None
                feed = []
                if c + 1 < NCHUNK:
                    hs_cur, feed = hs_dma_closures(c + 1)
                if c == 0:
                    feed.append(lambda: nc.sync.dma_start(woh_sb, woh_d.ap()))
                    feed.append(lambda: nc.sync.dma_start(wol_sb, wol_d.ap()))
                if c >= 1:
                    op = oproj_units(c - 1)
                    gi = 0
                    done_op = False
                    opn = 0
                    while not done_op:
                        if gi < len(tail_gens):
                            if drain(tail_gens[gi], 1):
                                gi += 1
                                continue
                        done_op = drain(op, 2 if qb else 4)
                        opn += 1
                        if feed and opn % 2 == 0:
                            feed.pop(0)()
                    for g in tail_gens[gi:]:
                        drain(g)
                else:
                    for f in feed:
                        f()
                    feed = []
                    for g in tail_gens:
                        drain(g)
                for f in feed:
                    f()
                if c == NCHUNK - 1 and pending[0] is not None:
                    # flush the last epilogue now: its DVE/Act ops run while
                    # the PE works through the final O projection below
                    epilogue(pending[0])
                    pending[0] = None
            drain(oproj_units(NCHUNK - 1))
    nc.compile()
    return nc


def _get_nc():
    if "nc" not in _CACHE:
        _CACHE["nc"] = _build()
    return _CACHE["nc"]


def _split8(x: np.ndarray):
    """Split f32 array into (hi, lo) e4m3 pair with hi + lo ~= x."""
    hi = x.astype(E4M3)
    lo = (x - hi.astype(np.float32)).astype(E4M3)
    return hi, lo


def _prep_inputs(inputs) -> list[dict]:
    bf16 = ml_dtypes.bfloat16
    hs = np.asarray(inputs["hidden_states"], dtype=np.float32).reshape(T, DIM)
    hsT = np.ascontiguousarray(hs.T)
    hsh, hsl = _split8(hsT)

    def swz_hs(x):  # [DIM, T] -> [128, NCHUNK, KT, CH] (SBUF layout)
        return np.ascontiguousarray(
            x.reshape(KT, 128, NCHUNK, CH).transpose(1, 2, 0, 3))

    hsh = swz_hs(hsh)
    hsl = swz_hs(hsl)

    fc = np.asarray(inputs["freqs_cos"], dtype=np.float32).reshape(T, HD // 2).T
    fs = np.asarray(inputs["freqs_sin"], dtype=np.float32).reshape(T, HD // 2).T
    cos2 = np.concatenate([fc, fc], axis=0)            # [128, T]
    sin2 = np.concatenate([-fs, fs], axis=0)           # signed half-rotation
    cos_qv = np.ascontiguousarray(cos2 * (SCALE / S_Q)).astype(bf16)
    sin_qv = np.ascontiguousarray(sin2 * (SCALE / S_Q)).astype(bf16)
    cos_kv = np.ascontiguousarray(cos2 * (1.0 / S_K)).astype(bf16)
    sin_kv = np.ascontiguousarray(sin2 * (1.0 / S_K)).astype(bf16)

    maskT = np.ascontiguousarray(
        np.asarray(inputs["attention_mask"], dtype=np.float32)[0, 0, :128, :128].T)

    perm = np.concatenate([np.arange(0, HD, 2), np.arange(1, HD, 2)])
    Wq = np.asarray(inputs["Wq"], dtype=np.float32)
    Wk = np.asarray(inputs["Wk"], dtype=np.float32)
    Wv = np.asarray(inputs["Wv"], dtype=np.float32)
    Wo = np.asarray(inputs["Wo"], dtype=np.float32)

    def swz_w(x, nh):  # [DIM, nh*HD] -> [128, nh, KT, HD]
        return np.ascontiguousarray(
            x.reshape(KT, 128, nh, HD).transpose(1, 2, 0, 3))

    in_maps = []
    for c in range(N_CORES):
        wq_c = np.concatenate(
            [Wq[:, (c * HL + h) * HD:(c * HL + h + 1) * HD][:, perm]
             for h in range(HL)], axis=1) * S_Q
        wk_c = Wk[:, c * HD:(c + 1) * HD][:, perm] * S_K
        wv_c = Wv[:, c * HD:(c + 1) * HD] * S_V
        wo_c = Wo[c * HL * HD:(c + 1) * HL * HD, :] * S_O
        wqh, wql = _split8(wq_c)
        wkh, wkl = _split8(wk_c)
        wvh, wvl = _split8(wv_c)
        woh, wol = _split8(wo_c)
        in_maps.append({
            "hsh": hsh, "hsl": hsl,
            "wqh": swz_w(wqh, HL), "wql": swz_w(wql, HL),
            "wkh": swz_w(wkh, 1).reshape(128, KT, HD),
            "wkl": swz_w(wkl, 1).reshape(128, KT, HD),
            "wvh": swz_w(wvh, 1).reshape(128, KT, HD),
            "wvl": swz_w(wvl, 1).reshape(128, KT, HD),
            "woh": np.ascontiguousarray(
                woh.reshape(HL, 128, DIM).transpose(1, 0, 2)),
            "wol": np.ascontiguousarray(
                wol.reshape(HL, 128, DIM).transpose(1, 0, 2)),
            "cos_q": cos_qv, "sin_q": sin_qv,
            "cos_k": cos_kv, "sin_k": sin_kv,
            "maskT": maskT,
        })
    return in_maps


def kernel(**inputs) -> np.ndarray:
    nc = _get_nc()
    in_maps = _prep_inputs(inputs)
    res = bass_utils.run_bass_kernel_spmd(nc, in_maps,
                                          core_ids=list(range(N_CORES)))
    acc = np.zeros((T, DIM), dtype=np.float32)
    for c in range(N_CORES):
        acc += np.asarray(res.results[c]["out"], dtype=np.float32)
    return (acc * (1.0 / (S_V * S_O))).reshape(B, S, DIM)
